# revision 1
# baseline (speedup 1.0000x reference)
"""Two-layer GCN (PyG GCNConv x2, relu between) on 8 trn2 NeuronCores.

Strategy (dst-node partitioned, all on-device math):
  - Nodes are sharded across 8 cores by destination row (12500/core).
  - Layer tables (dinv * (x@W1), then (dinv*relu(h))@W2-space inputs) are
    computed shard-wise on-device, AllGather'ed into a replicated DRAM table
    of 256B rows, and per-edge messages are fetched with GPSIMD dma_gather.
  - Segment-sum per 128-dst block is a PE matmul with a one-hot selection
    matrix built on DVE via is_equal against an iota row; PSUM accumulates
    across message chunks, so no scatter is needed.
  - Epilogues apply dinv/bias/relu and chain directly into the next layer's
    table transform. Final output is produced transposed and unsharded on
    host.

The Bass program is identical on all cores (SPMD); per-(block, src-group)
chunk counts are the max over cores, with padding slots pointing at a zero
table row.
"""

import math
import sys

sys.path.insert(0, "/opt/trn_rl_repo")

import numpy as np


# ---------------------------------------------------------------------------
# configuration
# ---------------------------------------------------------------------------
class Cfg:
    CORES = 8
    N = 100000
    IN_C = 128
    HID = 64
    OUT_C = 40
    NPC = 12500  # nodes per core
    NPC_PAD = 12544  # = 98 * 128
    BLK = 128
    SBB = 4  # dst blocks per superblock (gather-call granularity)
    GROUP_ROWS = 32768  # int16 index reach per gather call
    SG = 8  # chunks per S-build op
    MAXCH = 8  # max chunks per dma_gather call

    @property
    def NBLK(self):
        return self.NPC_PAD // self.BLK

    @property
    def NSB(self):
        return math.ceil(self.NBLK / self.SBB)

    @property
    def TAB(self):
        return self.NPC_PAD * self.CORES

    @property
    def NGRP(self):
        return math.ceil(self.TAB / self.GROUP_ROWS)


# ---------------------------------------------------------------------------
# host-side prep: shard edges, build shared static schedule + per-core arrays
# ---------------------------------------------------------------------------
def _prepare(cfg, edge_index):
    src = np.asarray(edge_index[0], dtype=np.int64)
    dst = np.asarray(edge_index[1], dtype=np.int64)
    loop = np.arange(cfg.N, dtype=np.int64)
    src = np.concatenate([src, loop])
    dst = np.concatenate([dst, loop])

    deg = np.bincount(dst, minlength=cfg.N).astype(np.float32)

    # zero table row per src-group (core pad rows are zero in both tables)
    zrow = []
    for g in range(cfg.NGRP):
        lo, hi = g * cfg.GROUP_ROWS, min((g + 1) * cfg.GROUP_ROWS, cfg.TAB)
        r = None
        for c in range(cfg.CORES):
            p0, p1 = c * cfg.NPC_PAD + cfg.NPC, (c + 1) * cfg.NPC_PAD
            a, b = max(p0, lo), min(p1, hi)
            if a < b:
                r = a
                break
        assert r is not None, f"no zero row available in src-group {g}"
        zrow.append(r)

    owner = dst // cfg.NPC
    dl_all = dst - owner * cfg.NPC
    srow_all = (src // cfg.NPC) * cfg.NPC_PAD + (src % cfg.NPC)
    grp_all = srow_all // cfg.GROUP_ROWS
    blk_all = dl_all // cfg.BLK

    per_core = []
    counts = np.zeros((cfg.CORES, cfg.NBLK, cfg.NGRP), dtype=np.int64)
    for c in range(cfg.CORES):
        m = owner == c
        srow, dl, grp, blk = srow_all[m], dl_all[m], grp_all[m], blk_all[m]
        # emission order: (superblock, group, block)
        key = (blk // cfg.SBB) * (cfg.NGRP * cfg.SBB) + grp * cfg.SBB + (blk % cfg.SBB)
        order = np.argsort(key, kind="stable")
        per_core.append((srow[order], dl[order], key[order]))
        np.add.at(counts[c], (blk, grp), 1)

    sched = np.ceil(counts.max(axis=0) / cfg.BLK).astype(np.int64)  # [NBLK, NGRP]

    # chunk sequence in emission order; calls = one dma_gather per (sb, g)
    chunk_blocks = []  # block id per chunk
    calls = []  # (g, first_chunk, n_chunks) per gather call
    for sb in range(cfg.NSB):
        blo, bhi = sb * cfg.SBB, min((sb + 1) * cfg.SBB, cfg.NBLK)
        for g in range(cfg.NGRP):
            nch = int(sched[blo:bhi, g].sum())
            if nch == 0:
                continue
            # split large calls so one SWDGE op can't overrun the desc ring
            done = 0
            while done < nch:
                take = min(cfg.MAXCH, nch - done)
                calls.append((g, len(chunk_blocks) + done, take))
                done += take
            for b in range(blo, bhi):
                chunk_blocks.extend([b] * int(sched[b, g]))
    nchunk = len(chunk_blocks)
    nslot = nchunk * cfg.BLK

    # per-core slot arrays following the shared schedule
    idx_maps = []
    dstloc_maps = []
    for c in range(cfg.CORES):
        srow, dl, key = per_core[c]
        idx_arr = np.empty(nslot, dtype=np.int64)
        dloc_arr = np.zeros(nslot, dtype=np.int64)
        pos = 0
        ei = 0
        for sb in range(cfg.NSB):
            blo, bhi = sb * cfg.SBB, min((sb + 1) * cfg.SBB, cfg.NBLK)
            for g in range(cfg.NGRP):
                for b in range(blo, bhi):
                    n = int(counts[c, b, g])
                    cap = int(sched[b, g]) * cfg.BLK
                    idx_arr[pos : pos + n] = srow[ei : ei + n] - g * cfg.GROUP_ROWS
                    dloc_arr[pos : pos + n] = dl[ei : ei + n] % cfg.BLK
                    idx_arr[pos + n : pos + cap] = zrow[g] - g * cfg.GROUP_ROWS
                    # pad dstloc stays 0 (gathers a zero row -> adds nothing)
                    pos += cap
                    ei += n
        assert pos == nslot and ei == len(srow)
        assert idx_arr.min() >= 0 and idx_arr.max() < cfg.GROUP_ROWS
        idx_maps.append(np.tile(idx_arr.astype(np.int16).reshape(-1, 16).T, (8, 1)))
        dstloc_maps.append(
            np.ascontiguousarray(dloc_arr.astype(np.float32).reshape(nchunk, cfg.BLK).T)
        )

    return {
        "deg": deg,
        "sched": sched,
        "chunk_blocks": chunk_blocks,
        "calls": calls,
        "nchunk": nchunk,
        "nslot": nslot,
        "idx_maps": idx_maps,
        "dstloc_maps": dstloc_maps,
    }


# ---------------------------------------------------------------------------
# device program
# ---------------------------------------------------------------------------
def _build(cfg, chunk_blocks, calls, debug=False, taps=False):
    import concourse.bacc as bacc
    import concourse.mybir as mybir
    import concourse.tile as tile
    from concourse import library_config

    fp32 = mybir.dt.float32
    bf16 = mybir.dt.bfloat16
    AF = mybir.ActivationFunctionType
    ALU = mybir.AluOpType

    nchunk = len(chunk_blocks)
    # first/last chunk per block (for PSUM start/stop flags)
    first_chunk = {}
    last_chunk = {}
    for j, b in enumerate(chunk_blocks):
        first_chunk.setdefault(b, j)
        last_chunk[b] = j
    max_call_ch = max(n for _, _, n in calls)

    nc = bacc.Bacc("TRN2", target_bir_lowering=False, debug=debug)

    xT_in = nc.dram_tensor("xT", [cfg.IN_C, cfg.NPC_PAD], fp32, kind="ExternalInput")
    W1_in = nc.dram_tensor("W1", [cfg.IN_C, cfg.HID], fp32, kind="ExternalInput")
    W2p_in = nc.dram_tensor("W2p", [cfg.HID, cfg.HID], fp32, kind="ExternalInput")
    b1_in = nc.dram_tensor("b1c", [cfg.HID, 1], fp32, kind="ExternalInput")
    b2_in = nc.dram_tensor("b2c", [cfg.OUT_C, 1], fp32, kind="ExternalInput")
    degnw_in = nc.dram_tensor(
        "deg_nw", [cfg.BLK, cfg.NBLK], fp32, kind="ExternalInput"
    )
    degfl_in = nc.dram_tensor("deg_flat", [1, cfg.NPC_PAD], fp32, kind="ExternalInput")
    idx_in = nc.dram_tensor(
        "idxs", [128, (nchunk * cfg.BLK) // 16, ], mybir.dt.int16, kind="ExternalInput"
    )
    dstloc_in = nc.dram_tensor(
        "dstloc", [cfg.BLK, nchunk], fp32, kind="ExternalInput"
    )
    out_t = nc.dram_tensor(
        "outT", [cfg.OUT_C, cfg.NPC_PAD], fp32, kind="ExternalOutput"
    )
    if taps:
        tap1 = nc.dram_tensor(
            "tap1", [cfg.NPC_PAD, cfg.HID], fp32, kind="ExternalOutput"
        )
        tap2 = nc.dram_tensor(
            "tap2", [cfg.NPC_PAD, cfg.HID], fp32, kind="ExternalOutput"
        )

    shard1 = nc.dram_tensor("shard1", [cfg.NPC_PAD, cfg.HID], fp32)
    shard2 = nc.dram_tensor("shard2", [cfg.NPC_PAD, cfg.HID], fp32)
    table1 = nc.dram_tensor(
        "table1", [cfg.TAB, cfg.HID], fp32, addr_space="Shared"
    )
    table2 = nc.dram_tensor(
        "table2", [cfg.TAB, cfg.HID], fp32, addr_space="Shared"
    )
    iota_c = nc.inline_tensor(
        np.tile(np.arange(cfg.BLK, dtype=np.float32), (128, cfg.SG)).reshape(
            128, cfg.SG * cfg.BLK
        ),
        name="iota_sg",
    )

    replica = [list(range(cfg.CORES))]

    with tile.TileContext(nc) as tc:
        with (
            tc.tile_pool(name="cst", bufs=1) as cst,
            tc.tile_pool(name="gp", bufs=3) as gp,
            tc.tile_pool(name="gbp", bufs=3) as gbp,
            tc.tile_pool(name="sp", bufs=4) as sp,
            tc.tile_pool(name="dv", bufs=2) as dv,
            tc.tile_pool(name="ev", bufs=6) as ev,
        ):
            nc.gpsimd.load_library(library_config.mlp)

            # ---- constants ----
            W1t = cst.tile([cfg.IN_C, cfg.HID], fp32)
            nc.sync.dma_start(W1t[:], W1_in[:])
            W2t = cst.tile([cfg.HID, cfg.HID], fp32)
            nc.sync.dma_start(W2t[:], W2p_in[:])
            b1t = cst.tile([cfg.HID, 1], fp32)
            nc.sync.dma_start(b1t[:], b1_in[:])
            b2t = cst.tile([cfg.OUT_C, 1], fp32)
            nc.sync.dma_start(b2t[:], b2_in[:])
            iota = cst.tile([128, cfg.SG * cfg.BLK], fp32)
            nc.sync.dma_start(iota[:], iota_c[:])
            degnw = cst.tile([cfg.BLK, cfg.NBLK], fp32)
            nc.sync.dma_start(degnw[:], degnw_in[:])
            dinvnw = cst.tile([cfg.BLK, cfg.NBLK], fp32)
            nc.scalar.activation(dinvnw[:], degnw[:], AF.Sqrt)
            nc.vector.reciprocal(dinvnw[:], dinvnw[:])
            idxt = cst.tile([128, (nchunk * cfg.BLK) // 16], mybir.dt.int16)
            nc.sync.dma_start(idxt[:], idx_in[:])
            dstloct = cst.tile([cfg.BLK, nchunk], fp32)
            nc.sync.dma_start(dstloct[:], dstloc_in[:])

            # ---- layer-1 transform: shard1 = dinv * (x @ W1), row-major ----
            with (
                tc.tile_pool(name="phA", bufs=3) as pa,
                tc.tile_pool(name="stg1", bufs=1) as st1,
                tc.tile_pool(name="psA", bufs=4, space="PSUM") as psA,
            ):
                stag1 = st1.tile([cfg.BLK, cfg.NBLK, cfg.HID], fp32)
                for b in range(cfg.NBLK):
                    xc = pa.tile([cfg.IN_C, cfg.BLK], fp32)
                    nc.sync.dma_start(
                        xc[:], xT_in[:, b * cfg.BLK : (b + 1) * cfg.BLK]
                    )
                    ps = psA.tile([cfg.BLK, cfg.HID], fp32)
                    nc.tensor.matmul(ps[:], lhsT=xc[:], rhs=W1t[:], start=True, stop=True)
                    nc.vector.tensor_scalar(
                        out=stag1[:, b, :],
                        in0=ps[:],
                        scalar1=dinvnw[:, b : b + 1],
                        scalar2=None,
                        op0=ALU.mult,
                    )
                nc.sync.dma_start(
                    shard1.rearrange("(b p) d -> p b d", p=cfg.BLK)[:], stag1[:]
                )

            nc.gpsimd.collective_compute(
                "AllGather",
                mybir.AluOpType.bypass,
                replica_groups=replica,
                ins=[shard1[:]],
                outs=[table1[:]],
            )
            if taps:
                nc.sync.dma_start(tap1[:], shard1[:])

            # ---- aggregation layer (shared for both layers) ----
            def agg_layer(layer, table, stag2):
                ch_out = cfg.HID if layer == 1 else cfg.OUT_C
                # S tiles for the whole chunk sequence, built in groups of SG
                s_tiles = {}

                def s_for(j):
                    gi = j // cfg.SG
                    if gi not in s_tiles:
                        n = min(cfg.SG, nchunk - gi * cfg.SG)
                        st = sp.tile([128, cfg.SG * cfg.BLK], bf16, tag="s")
                        nc.vector.tensor_tensor(
                            out=st[:].rearrange("p (a b) -> p a b", b=cfg.BLK)[
                                :, :n, :
                            ],
                            in0=iota[:].rearrange("p (a b) -> p a b", b=cfg.BLK)[
                                :, :n, :
                            ],
                            in1=dstloct[:, gi * cfg.SG : gi * cfg.SG + n].to_broadcast(
                                [128, n, cfg.BLK]
                            ),
                            op=ALU.is_equal,
                        )
                        s_tiles[gi] = st
                    return s_tiles[gi], (j % cfg.SG)

                psums = {}
                ci = 0  # call cursor
                for sb in range(cfg.NSB):
                    blo, bhi = sb * cfg.SBB, min((sb + 1) * cfg.SBB, cfg.NBLK)
                    # dinv replicated across partitions for this superblock
                    nsb = (bhi - blo) * cfg.BLK
                    degsl = dv.tile([1, cfg.SBB * cfg.BLK], fp32, tag="ds")
                    nc.sync.dma_start(
                        degsl[:, :nsb],
                        degfl_in[:, blo * cfg.BLK : blo * cfg.BLK + nsb],
                    )
                    degrep = dv.tile([128, cfg.SBB * cfg.BLK], fp32, tag="dg")
                    nc.gpsimd.partition_broadcast(degrep[:, :nsb], degsl[:, :nsb])
                    dinvrep = dv.tile([128, cfg.SBB * cfg.BLK], fp32, tag="dr")
                    nc.scalar.activation(dinvrep[:, :nsb], degrep[:, :nsb], AF.Sqrt)
                    nc.vector.reciprocal(dinvrep[:, :nsb], dinvrep[:, :nsb])

                    # gather + matmul-aggregate this superblock's calls
                    while ci < len(calls):
                        g, j0, nch = calls[ci]
                        if chunk_blocks[j0] >= bhi:
                            break
                        ci += 1
                        base = g * cfg.GROUP_ROWS
                        rows = min(cfg.GROUP_ROWS, cfg.TAB - base)
                        gt = gp.tile([128, max_call_ch, cfg.HID], fp32, tag="g")
                        nc.gpsimd.dma_gather(
                            gt[:, :nch, :],
                            table[base : base + rows, :],
                            idxt[:, (j0 * cfg.BLK) // 16 : ((j0 + nch) * cfg.BLK) // 16],
                            nch * cfg.BLK,
                            nch * cfg.BLK,
                            cfg.HID,
                        )
                        gbt = gbp.tile(
                            [128, max_call_ch, cfg.HID], bf16, tag="gb"
                        )
                        nc.scalar.activation(
                            gbt[:, :nch, :], gt[:, :nch, :], AF.Copy
                        )
                        for j in range(j0, j0 + nch):
                            b = chunk_blocks[j]
                            if b not in psums:
                                pstile = tc_psum.tile(
                                    [ch_out, cfg.BLK], fp32, tag=f"ps{layer}"
                                )
                                psums[b] = pstile
                            st, k = s_for(j)
                            nc.tensor.matmul(
                                psums[b][:],
                                lhsT=gbt[:, j - j0, :ch_out],
                                rhs=st[:, k * cfg.BLK : (k + 1) * cfg.BLK],
                                start=(j == first_chunk[b]),
                                stop=(j == last_chunk[b]),
                            )

                    # epilogues for this superblock's blocks
                    for b in range(blo, bhi):
                        off = (b - blo) * cfg.BLK
                        ps = psums.pop(b)
                        if layer == 1:
                            t1 = ev.tile([cfg.HID, cfg.BLK], fp32, tag="t1")
                            nc.vector.tensor_tensor(
                                out=t1[:],
                                in0=ps[:],
                                in1=dinvrep[: cfg.HID, off : off + cfg.BLK],
                                op=ALU.mult,
                            )
                            hr = ev.tile([cfg.HID, cfg.BLK], fp32, tag="hr")
                            nc.scalar.activation(hr[:], t1[:], AF.Relu, bias=b1t[:])
                            gb = ev.tile([cfg.HID, cfg.BLK], fp32, tag="gblk")
                            nc.vector.tensor_tensor(
                                out=gb[:],
                                in0=hr[:],
                                in1=dinvrep[: cfg.HID, off : off + cfg.BLK],
                                op=ALU.mult,
                            )
                            ps2 = tc_ps2.tile([cfg.BLK, cfg.HID], fp32, tag="ps2")
                            nc.tensor.matmul(
                                ps2[:], lhsT=gb[:], rhs=W2t[:], start=True, stop=True
                            )
                            nc.vector.tensor_copy(stag2[:, b, :], ps2[:])
                        else:
                            t1 = ev.tile([cfg.OUT_C, cfg.BLK], fp32, tag="t2")
                            nc.vector.tensor_tensor(
                                out=t1[:],
                                in0=ps[:],
                                in1=dinvrep[: cfg.OUT_C, off : off + cfg.BLK],
                                op=ALU.mult,
                            )
                            nc.vector.tensor_scalar(
                                out=stag2[:, b * cfg.BLK : (b + 1) * cfg.BLK],
                                in0=t1[:],
                                scalar1=b2t[:],
                                scalar2=None,
                                op0=ALU.add,
                            )

            # layer 1 aggregation (+ table2 transform fused in epilogue)
            with (
                tc.tile_pool(name="stg2", bufs=1) as st2p,
                tc.tile_pool(name="ps2p", bufs=2, space="PSUM") as tc_ps2,
                tc.tile_pool(name="psagg1", bufs=6, space="PSUM") as tc_psum,
            ):
                stag2 = st2p.tile([cfg.BLK, cfg.NBLK, cfg.HID], fp32)
                agg_layer(1, table1, stag2)
                nc.sync.dma_start(
                    shard2.rearrange("(b p) d -> p b d", p=cfg.BLK)[:], stag2[:]
                )

            nc.gpsimd.collective_compute(
                "AllGather",
                mybir.AluOpType.bypass,
                replica_groups=replica,
                ins=[shard2[:]],
                outs=[table2[:]],
            )
            if taps:
                nc.sync.dma_start(tap2[:], shard2[:])

            # layer 2 aggregation -> transposed output
            with (
                tc.tile_pool(name="outp", bufs=1) as outp,
                tc.tile_pool(name="psagg2", bufs=6, space="PSUM") as tc_psum,
                tc.tile_pool(name="ps2p2", bufs=1, space="PSUM") as tc_ps2,
            ):
                outT = outp.tile([cfg.OUT_C, cfg.NPC_PAD], fp32)
                agg_layer(2, table2, outT)
                nc.sync.dma_start(out_t[:], outT[:])

    nc.compile()
    return nc


# ---------------------------------------------------------------------------
# public entry point
# ---------------------------------------------------------------------------
def _make_in_maps(cfg, prep, x, W1, b1, W2, b2):
    W2p = np.zeros((cfg.HID, cfg.HID), np.float32)
    W2p[:, : cfg.OUT_C] = W2
    deg = prep["deg"]
    in_maps = []
    for c in range(cfg.CORES):
        xs = x[c * cfg.NPC : (c + 1) * cfg.NPC]  # [NPC, IN_C]
        xT = np.zeros((cfg.IN_C, cfg.NPC_PAD), np.float32)
        xT[:, : cfg.NPC] = xs.T
        # pad nodes: huge degree -> dinv ~ 1e-19 -> pad table rows ~ 0
        dg = np.full(cfg.NPC_PAD, 1e30, np.float32)
        dg[: cfg.NPC] = deg[c * cfg.NPC : (c + 1) * cfg.NPC]
        deg_nw = np.ascontiguousarray(dg.reshape(cfg.NBLK, cfg.BLK).T)
        in_maps.append(
            {
                "xT": xT,
                "W1": np.asarray(W1, np.float32),
                "W2p": W2p,
                "b1c": np.asarray(b1, np.float32).reshape(cfg.HID, 1),
                "b2c": np.asarray(b2, np.float32).reshape(cfg.OUT_C, 1),
                "deg_nw": deg_nw,
                "deg_flat": dg.reshape(1, cfg.NPC_PAD),
                "idxs": prep["idx_maps"][c],
                "dstloc": prep["dstloc_maps"][c],
            }
        )
    return in_maps


def _run(cfg, inputs, mode="hw", trace=False, taps=False):
    x = np.asarray(inputs["x"], np.float32)
    edge_index = np.asarray(inputs["edge_index"])
    W1 = np.asarray(inputs["W1"], np.float32)
    b1 = np.asarray(inputs["b1"], np.float32)
    W2 = np.asarray(inputs["W2"], np.float32)
    b2 = np.asarray(inputs["b2"], np.float32)

    prep = _prepare(cfg, edge_index)
    nc = _build(cfg, prep["chunk_blocks"], prep["calls"], debug=(mode == "sim"), taps=taps)
    in_maps = _make_in_maps(cfg, prep, x, W1, b1, W2, b2)

    info = {}
    if mode == "sim":
        from concourse.bass_interp import MultiCoreSim

        sim = MultiCoreSim(nc, cfg.CORES)
        for c in range(cfg.CORES):
            for k, v in in_maps[c].items():
                sim.cores[c].tensor(k)[:] = v
        sim.simulate()
        outs = [sim.cores[c].tensor("outT").copy() for c in range(cfg.CORES)]
        if taps:
            info["taps"] = [
                {k: sim.cores[c].tensor(k).copy() for k in ("tap1", "tap2")}
                for c in range(cfg.CORES)
            ]
    else:
        import concourse.bass_utils as bu

        if trace:
            # avoid the S3 artifact upload in the profile path
            bu.upload_artifacts = lambda d: "(local)"
        r = bu.run_bass_kernel_spmd(
            nc, in_maps, list(range(cfg.CORES)), trace=trace,
            tmpdir=(inputs.get("_tracedir") if trace else None),
        )
        info["exec_time_ns"] = r.exec_time_ns
        info["mean_exec_time_ns"] = r.mean_exec_time_ns
        outs = [r.results[c]["outT"] for c in range(cfg.CORES)]
        if taps:
            info["taps"] = [
                {k: r.results[c][k] for k in ("tap1", "tap2")}
                for c in range(cfg.CORES)
            ]

    out = np.concatenate([o[:, : cfg.NPC].T for o in outs], axis=0)
    return out.astype(np.float32), info


def kernel(**inputs):
    out, _ = _run(Cfg(), inputs, mode="hw")
    return out



# revision 2
# speedup vs baseline: 1.0255x; 1.0255x over previous
"""Two-layer GCN (PyG GCNConv x2, relu between) on 8 trn2 NeuronCores. v2.

Strategy (dst-node partitioned, all on-device math):
  - Nodes sharded across 8 cores by destination row (12500/core).
  - Per-layer node tables (dinv * (x@W1), then dinv*relu(h1)) are computed
    shard-wise, AllGather'ed into a replicated bf16 DRAM table of 128B rows,
    and per-edge messages fetched with GPSIMD dma_gather at pair-row (256B)
    granularity (two nodes per gathered row; parity picks the half).
  - Aggregation = PE matmul with one-hot S as the *stationary* operand
    (FWL path): psum[dst,hid] += S[msg,dst]^T @ G[msg,hid].  S built on DVE
    via is_equal against an iota row; 255-sentinel slots contribute nothing,
    letting chunks span dst blocks (segment-packed schedule, less padding).
  - Self-loops are excluded from the edge stream and added with one identity
    matmul per block reading the local stag tile.
  - Epilogues use per-partition dinv columns (tensor_scalar); W2 is applied
    in the layer-2 epilogue via a PE transpose per block.
"""

import math
import sys

sys.path.insert(0, "/opt/trn_rl_repo")

import numpy as np
import ml_dtypes

BF16 = ml_dtypes.bfloat16


class Cfg:
    CORES = 8
    N = 100000
    IN_C = 128
    HID = 64
    OUT_C = 40
    NPC = 12500
    NPC_PAD = 12544  # 98*128
    BLK = 128
    SBB = 6  # dst blocks per superblock (PSUM: 8 banks; SBB + pt + po <= 8)
    GROUP_PAIRS = 32768  # int16 index reach per gather call (pair rows)
    SG = 8  # tasks per S-build op
    MAXCH = 8  # max chunks per dma_gather call (big calls crash NRT)
    NQ = 4  # SWDGE queues (desc-gen parallelism)

    @property
    def NBLK(self):
        return self.NPC_PAD // self.BLK

    @property
    def NSB(self):
        return math.ceil(self.NBLK / self.SBB)

    @property
    def TAB(self):
        return self.NPC_PAD * self.CORES

    @property
    def NGRP(self):
        return math.ceil((self.TAB // 2) / self.GROUP_PAIRS)


# ---------------------------------------------------------------------------
# host-side prep: shard edges, build shared static schedule + per-core arrays
# ---------------------------------------------------------------------------
def _prepare(cfg, edge_index):
    src = np.asarray(edge_index[0], dtype=np.int64)
    dst = np.asarray(edge_index[1], dtype=np.int64)

    deg = np.bincount(dst, minlength=cfg.N).astype(np.float32) + 1.0  # + self loop
    dinv = 1.0 / np.sqrt(deg)

    owner = dst // cfg.NPC
    dl_all = dst - owner * cfg.NPC
    blk_all = dl_all // cfg.BLK
    sb_all = blk_all // cfg.SBB
    srow_all = (src // cfg.NPC) * cfg.NPC_PAD + (src % cfg.NPC)
    pair_all = srow_all >> 1
    par_all = srow_all & 1
    grp_all = pair_all // cfg.GROUP_PAIRS

# per-core sorted edge streams and per-(cell, block) counts
    per_core = []
    counts = np.zeros(
        (cfg.CORES, cfg.NSB, cfg.NGRP, 2, cfg.NBLK), dtype=np.int64
    )
    for c in range(cfg.CORES):
        m = owner == c
        pr, pa, g, b, dl, s = (
            pair_all[m],
            par_all[m],
            grp_all[m],
            blk_all[m],
            dl_all[m],
            sb_all[m],
        )
        key = ((s * cfg.NGRP + g) * 2 + pa) * cfg.NBLK + b
        order = np.argsort(key, kind="stable")
        per_core.append((pr[order], dl[order], key[order]))
        np.add.at(counts[c], (s, g, pa, b), 1)

    segmax = counts.max(axis=0)  # [NSB, NGRP, 2, NBLK] max msgs per segment

    # shared static schedule:
    #   cells iterate (sb, grp, par); within a cell, block segments of length
    #   segmax are packed back to back, cell padded to a chunk multiple.
    calls = []  # (sb, grp, par, first_chunk, nch) per dma_gather call
    tasks = []  # (chunk, block) in emission order
    chunk_tasks = []  # per chunk: list of task ids
    cell_meta = []  # (sb, grp, par, slot0, seg_offsets, nch)
    nslot = 0
    nchunk = 0
    for sb in range(cfg.NSB):
        blo = sb * cfg.SBB
        bhi = min((sb + 1) * cfg.SBB, cfg.NBLK)
        for g in range(cfg.NGRP):
            for pa in range(2):
                segs = segmax[sb, g, pa, blo:bhi]
                tot = int(segs.sum())
                nch = max(1, math.ceil(tot / cfg.BLK))
                offs = np.concatenate([[0], np.cumsum(segs)])
                cell_meta.append((sb, g, pa, nslot, offs, nch))
                # tasks: (chunk, block) for every chunk x overlapping segment
                for j in range(nch):
                    chunk_tasks.append([])
                    s0, s1 = j * cfg.BLK, (j + 1) * cfg.BLK
                    for bi in range(bhi - blo):
                        if offs[bi] < s1 and offs[bi + 1] > s0 and segs[bi] > 0:
                            chunk_tasks[-1].append(len(tasks))
                            tasks.append((nchunk + j, blo + bi))
                # gather calls (split evenly, each <= MAXCH chunks)
                npart = math.ceil(nch / cfg.MAXCH)
                done = 0
                for ki in range(npart):
                    take = (nch - done + npart - ki - 1) // (npart - ki)
                    calls.append((sb, g, pa, nchunk + done, take))
                    done += take
                nslot += nch * cfg.BLK
                nchunk += nch

    ntask = len(tasks)

    # per-core slot arrays (idx + per-task dstloc columns)
    idx_maps = []
    dstloc_maps = []
    for c in range(cfg.CORES):
        pr, dl, key = per_core[c]
        idx_arr = np.zeros(nslot, dtype=np.int64)
        dloc_arr = np.full(nslot, 255, dtype=np.int64)
        dblk_arr = np.full(nslot, -1, dtype=np.int64)
        pos = 0
        ei = 0
        for sb, g, pa, slot0, offs, nch in cell_meta:
            blo = sb * cfg.SBB
            bhi = min((sb + 1) * cfg.SBB, cfg.NBLK)
            for bi in range(bhi - blo):
                n = int(counts[c, sb, g, pa, blo + bi])
                p0 = slot0 + int(offs[bi])
                idx_arr[p0 : p0 + n] = pr[ei : ei + n] - g * cfg.GROUP_PAIRS
                dloc_arr[p0 : p0 + n] = dl[ei : ei + n] % cfg.BLK
                dblk_arr[p0 : p0 + n] = blo + bi
                ei += n
            pos = slot0 + nch * cfg.BLK
        assert ei == len(pr)
        assert idx_arr.min() >= 0 and idx_arr.max() < cfg.GROUP_PAIRS
        idx_maps.append(np.tile(idx_arr.astype(np.int16).reshape(-1, 16).T, (8, 1)))
        # per-task dstloc columns: dl%128 where the slot's block == task block
        dl_mat = dloc_arr.reshape(nchunk, cfg.BLK)
        db_mat = dblk_arr.reshape(nchunk, cfg.BLK)
        dt = np.full((ntask, cfg.BLK), 255, dtype=np.int64)
        for t, (j, b) in enumerate(tasks):
            sel = db_mat[j] == b
            dt[t, sel] = dl_mat[j, sel]
        dstloc_maps.append(np.ascontiguousarray(dt.astype(np.float32).T))  # [128, ntask]

    return {
        "dinv": dinv,
        "calls": calls,
        "tasks": tasks,
        "chunk_tasks": chunk_tasks,
        "cell_meta": cell_meta,
        "nchunk": nchunk,
        "nslot": nslot,
        "ntask": ntask,
        "idx_maps": idx_maps,
        "dstloc_maps": dstloc_maps,
    }


# ---------------------------------------------------------------------------
# device program
# ---------------------------------------------------------------------------
def _build(cfg, prep, debug=False, taps=False):
    import concourse.bacc as bacc
    import concourse.mybir as mybir
    import concourse.tile as tile
    from concourse import library_config

    fp32 = mybir.dt.float32
    bf16 = mybir.dt.bfloat16
    AF = mybir.ActivationFunctionType
    ALU = mybir.AluOpType

    calls = prep["calls"]
    tasks = prep["tasks"]
    chunk_tasks = prep["chunk_tasks"]
    nchunk = prep["nchunk"]
    nslot = prep["nslot"]
    ntask = prep["ntask"]

    # last task index per block (for PSUM stop flags)
    last_task_of_blk = {}
    for t, (j, b) in enumerate(tasks):
        last_task_of_blk[b] = t
    max_call_ch = max(n for _, _, _, _, n in calls)

    nc = bacc.Bacc("TRN2", target_bir_lowering=False, debug=debug, num_swdge_queues=cfg.NQ, dynamic_dma_scratch_size=32768)

    xT_in = nc.dram_tensor("xT", [cfg.IN_C, cfg.NPC_PAD], bf16, kind="ExternalInput")
    W1_in = nc.dram_tensor("W1", [cfg.IN_C, cfg.HID], bf16, kind="ExternalInput")
    W2_in = nc.dram_tensor("W2", [cfg.HID, cfg.OUT_C], bf16, kind="ExternalInput")
    b1r_in = nc.dram_tensor("b1rep", [cfg.BLK, cfg.HID], fp32, kind="ExternalInput")
    b2c_in = nc.dram_tensor("b2col", [cfg.OUT_C, 1], fp32, kind="ExternalInput")
    dinv_in = nc.dram_tensor("dinvcol", [cfg.BLK, cfg.NBLK], fp32, kind="ExternalInput")
    ident_in = nc.dram_tensor("ident", [cfg.BLK, cfg.BLK], bf16, kind="ExternalInput")
    iota_in = nc.dram_tensor("iota", [cfg.BLK, cfg.SG * cfg.BLK], fp32, kind="ExternalInput")
    idx_in = nc.dram_tensor("idxs", [128, nslot // 16], mybir.dt.int16, kind="ExternalInput")
    dstloc_in = nc.dram_tensor("dstloc", [cfg.BLK, ntask], fp32, kind="ExternalInput")
    out_t = nc.dram_tensor("outT", [cfg.OUT_C, cfg.NPC_PAD], bf16, kind="ExternalOutput")
    if taps:
        tap1 = nc.dram_tensor("tap1", [cfg.NPC_PAD, cfg.HID], bf16, kind="ExternalOutput")
        tap2 = nc.dram_tensor("tap2", [cfg.NPC_PAD, cfg.HID], bf16, kind="ExternalOutput")

    shard1 = nc.dram_tensor("shard1", [cfg.NPC_PAD, cfg.HID], bf16)
    shard2 = nc.dram_tensor("shard2", [cfg.NPC_PAD, cfg.HID], bf16)
    table1 = nc.dram_tensor("table1", [cfg.TAB, cfg.HID], bf16, addr_space="Shared")
    table2 = nc.dram_tensor("table2", [cfg.TAB, cfg.HID], bf16, addr_space="Shared")

    replica = [list(range(cfg.CORES))]
    npairs = cfg.TAB // 2

    with tile.TileContext(nc) as tc:
        with (
            tc.tile_pool(name="cst", bufs=1) as cst,
            tc.tile_pool(name="stg", bufs=1) as stg,
            tc.tile_pool(name="gp", bufs=10) as gp,
            tc.tile_pool(name="sp", bufs=10) as sp,
            tc.tile_pool(name="ev", bufs=6) as ev,
        ):
            nc.gpsimd.load_library(library_config.mlp)

            # ---- constants ----
            W1t = cst.tile([cfg.IN_C, cfg.HID], bf16)
            nc.sync.dma_start(W1t[:], W1_in[:])
            W2t = cst.tile([cfg.HID, cfg.OUT_C], bf16)
            nc.sync.dma_start(W2t[:], W2_in[:])
            b1rt = cst.tile([cfg.BLK, cfg.HID], fp32)
            nc.sync.dma_start(b1rt[:], b1r_in[:])
            b2t = cst.tile([cfg.OUT_C, 1], fp32)
            nc.sync.dma_start(b2t[:], b2c_in[:])
            dinvt = cst.tile([cfg.BLK, cfg.NBLK], fp32)
            nc.sync.dma_start(dinvt[:], dinv_in[:])
            identt = cst.tile([cfg.BLK, cfg.BLK], bf16)
            nc.sync.dma_start(identt[:], ident_in[:])
            iota = cst.tile([cfg.BLK, cfg.SG * cfg.BLK], fp32)
            nc.sync.dma_start(iota[:], iota_in[:])
            idxt = cst.tile([128, nslot // 16], mybir.dt.int16)
            nc.sync.dma_start(idxt[:], idx_in[:])
            dstloct = cst.tile([cfg.BLK, ntask], fp32)
            nc.sync.dma_start(dstloct[:], dstloc_in[:])

            stag1 = stg.tile([cfg.BLK, cfg.NBLK, cfg.HID], bf16)
            stag2 = stg.tile([cfg.BLK, cfg.NBLK, cfg.HID], bf16)
            outT = stg.tile([cfg.OUT_C, cfg.NPC_PAD], bf16)

            # ---- layer-1 transform: stag1 = dinv * (x @ W1) ----
            with (
                tc.tile_pool(name="phA", bufs=3) as pa,
                tc.tile_pool(name="psA", bufs=4, space="PSUM") as psA,
            ):
                for b in range(cfg.NBLK):
                    xc = pa.tile([cfg.IN_C, cfg.BLK], bf16)
                    nc.sync.dma_start(xc[:], xT_in[:, b * cfg.BLK : (b + 1) * cfg.BLK])
                    ps = psA.tile([cfg.BLK, cfg.HID], fp32)
                    nc.tensor.matmul(ps[:], lhsT=xc[:], rhs=W1t[:], start=True, stop=True)
                    nc.vector.tensor_scalar(
                        out=stag1[:, b, :],
                        in0=ps[:],
                        scalar1=dinvt[:, b : b + 1],
                        scalar2=None,
                        op0=ALU.mult,
                    )
                nc.sync.dma_start(
                    shard1.rearrange("(b p) d -> p b d", p=cfg.BLK)[:], stag1[:]
                )

            nc.gpsimd.collective_compute(
                "AllGather",
                mybir.AluOpType.bypass,
                replica_groups=replica,
                ins=[shard1[:]],
                outs=[table1[:]],
            )
            if taps:
                nc.sync.dma_start(tap1[:], shard1[:])

            # ---- aggregation (shared for both layers) ----
            def agg_layer(layer, table, stag_src, epilogue):
                pairs = table.rearrange("(r two) d -> r (two d)", two=2)
                s_tiles = {}

                def s_for(t):
                    gi = t // cfg.SG
                    if gi not in s_tiles:
                        n = min(cfg.SG, ntask - gi * cfg.SG)
                        st = sp.tile([128, cfg.SG * cfg.BLK], bf16, tag="s")
                        nc.vector.tensor_tensor(
                            out=st[:].rearrange("p (a b) -> p a b", b=cfg.BLK)[:, :n, :],
                            in0=iota[:].rearrange("p (a b) -> p a b", b=cfg.BLK)[
                                :, :n, :
                            ],
                            in1=dstloct[:, gi * cfg.SG : gi * cfg.SG + n].to_broadcast(
                                [128, n, cfg.BLK]
                            ),
                            op=ALU.is_equal,
                        )
                        s_tiles[gi] = st
                    return s_tiles[gi], (t % cfg.SG)

                for sb in range(cfg.NSB):
                    blo = sb * cfg.SBB
                    bhi = min((sb + 1) * cfg.SBB, cfg.NBLK)
                    psums = {}
                    for b in range(blo, bhi):
                        ps = tc_psum.tile([cfg.BLK, cfg.HID], fp32, tag=f"ps{layer}")
                        psums[b] = ps
                        nc.tensor.matmul(
                            ps[:],
                            lhsT=identt[:],
                            rhs=stag_src[:, b, :],
                            start=True,
                            stop=(last_task_of_blk.get(b) is None),
                        )
                    # gather + aggregate this superblock's calls
                    for ci, (call_sb, g, pa_, j0, nch) in enumerate(calls):
                        if call_sb != sb:
                            continue
                        base = g * cfg.GROUP_PAIRS
                        rows = min(cfg.GROUP_PAIRS, npairs - base)
                        gt = gp.tile([128, max_call_ch, 2 * cfg.HID], bf16, tag="g")
                        nc.gpsimd.dma_gather(
                            gt[:, :nch, :],
                            pairs[base : base + rows, :],
                            idxt[:, (j0 * cfg.BLK) // 16 : ((j0 + nch) * cfg.BLK) // 16],
                            nch * cfg.BLK,
                            nch * cfg.BLK,
                            2 * cfg.HID,
                            queue_num=ci % cfg.NQ,
                        )
                        half = slice(pa_ * cfg.HID, (pa_ + 1) * cfg.HID)
                        for j in range(j0, j0 + nch):
                            for t in chunk_tasks[j]:
                                _, b = tasks[t]
                                st, k = s_for(t)
                                nc.tensor.matmul(
                                    psums[b][:],
                                    lhsT=st[:, k * cfg.BLK : (k + 1) * cfg.BLK],
                                    rhs=gt[:, j - j0, half],
                                    start=False,
                                    stop=(t == last_task_of_blk[b]),
                                )
                    for b in range(blo, bhi):
                        epilogue(b, psums.pop(b))

            # layer 1 aggregation
            def epi1(b, ps):
                u = ev.tile([cfg.BLK, cfg.HID], fp32, tag="u")
                nc.vector.tensor_scalar(
                    out=u[:], in0=ps[:], scalar1=dinvt[:, b : b + 1], scalar2=None,
                    op0=ALU.mult,
                )
                v = ev.tile([cfg.BLK, cfg.HID], fp32, tag="v")
                nc.vector.tensor_tensor(out=v[:], in0=u[:], in1=b1rt[:], op=ALU.add)
                w = ev.tile([cfg.BLK, cfg.HID], fp32, tag="w")
                nc.scalar.activation(w[:], v[:], AF.Relu)
                nc.vector.tensor_scalar(
                    out=stag2[:, b, :], in0=w[:], scalar1=dinvt[:, b : b + 1],
                    scalar2=None, op0=ALU.mult,
                )

            with tc.tile_pool(name="psagg1", bufs=cfg.SBB, space="PSUM") as tc_psum:
                agg_layer(1, table1, stag1, epi1)

            nc.sync.dma_start(
                shard2.rearrange("(b p) d -> p b d", p=cfg.BLK)[:], stag2[:]
            )
            nc.gpsimd.collective_compute(
                "AllGather",
                mybir.AluOpType.bypass,
                replica_groups=replica,
                ins=[shard2[:]],
                outs=[table2[:]],
            )
            if taps:
                nc.sync.dma_start(tap2[:], shard2[:])

            # layer 2 aggregation -> transposed output
            def epi2(b, ps):
                z = ev.tile([cfg.BLK, cfg.HID], bf16, tag="z")
                nc.vector.tensor_scalar(
                    out=z[:], in0=ps[:], scalar1=dinvt[:, b : b + 1], scalar2=None,
                    op0=ALU.mult,
                )
                pst = tc_pt.tile([cfg.HID, cfg.BLK], fp32, tag="pt")
                nc.tensor.matmul(pst[:], lhsT=z[:], rhs=identt[:], start=True, stop=True)
                zt = ev.tile([cfg.HID, cfg.BLK], bf16, tag="zt")
                nc.vector.tensor_copy(zt[:], pst[:])
                pso = tc_po.tile([cfg.OUT_C, cfg.BLK], fp32, tag="po")
                nc.tensor.matmul(pso[:], lhsT=W2t[:], rhs=zt[:], start=True, stop=True)
                nc.vector.tensor_scalar(
                    out=outT[:, b * cfg.BLK : (b + 1) * cfg.BLK], in0=pso[:],
                    scalar1=b2t[:], scalar2=None, op0=ALU.add,
                )

            with (
                tc.tile_pool(name="psagg2", bufs=cfg.SBB, space="PSUM") as tc_psum,
                tc.tile_pool(name="pspt", bufs=1, space="PSUM") as tc_pt,
                tc.tile_pool(name="pspo", bufs=1, space="PSUM") as tc_po,
            ):
                agg_layer(2, table2, stag2, epi2)

            nc.sync.dma_start(out_t[:], outT[:])

    nc.compile()
    return nc


# ---------------------------------------------------------------------------
# public entry point
# ---------------------------------------------------------------------------
def _make_in_maps(cfg, prep, x, W1, b1, W2, b2):
    dinv = prep["dinv"]
    iota = np.tile(np.arange(cfg.BLK, dtype=np.float32), (cfg.BLK, cfg.SG)).reshape(
        cfg.BLK, cfg.SG * cfg.BLK
    )
    in_maps = []
    for c in range(cfg.CORES):
        xs = np.asarray(x[c * cfg.NPC : (c + 1) * cfg.NPC], np.float32)
        xT = np.zeros((cfg.IN_C, cfg.NPC_PAD), np.float32)
        xT[:, : cfg.NPC] = xs.T
        dv = np.zeros(cfg.NPC_PAD, np.float32)
        dv[: cfg.NPC] = dinv[c * cfg.NPC : (c + 1) * cfg.NPC]
        in_maps.append(
            {
                "xT": xT.astype(BF16),
                "W1": np.asarray(W1, np.float32).astype(BF16),
                "W2": np.asarray(W2, np.float32).astype(BF16),
                "b1rep": np.tile(np.asarray(b1, np.float32), (cfg.BLK, 1)),
                "b2col": np.asarray(b2, np.float32).reshape(cfg.OUT_C, 1),
                "dinvcol": np.ascontiguousarray(
                    dv.reshape(cfg.NBLK, cfg.BLK).T
                ),
                "ident": np.eye(cfg.BLK, dtype=np.float32).astype(BF16),
                "iota": iota.astype(np.float32),
                "idxs": prep["idx_maps"][c],
                "dstloc": prep["dstloc_maps"][c],
            }
        )
    return in_maps


def _run(cfg, inputs, mode="hw", trace=False, taps=False):
    x = np.asarray(inputs["x"], np.float32)
    edge_index = np.asarray(inputs["edge_index"])
    W1 = np.asarray(inputs["W1"], np.float32)
    b1 = np.asarray(inputs["b1"], np.float32)
    W2 = np.asarray(inputs["W2"], np.float32)
    b2 = np.asarray(inputs["b2"], np.float32)

    prep = _prepare(cfg, edge_index)
    nc = _build(cfg, prep, debug=(mode == "sim"), taps=taps)
    in_maps = _make_in_maps(cfg, prep, x, W1, b1, W2, b2)

    info = {}
    if mode == "sim":
        from concourse.bass_interp import MultiCoreSim

        sim = MultiCoreSim(nc, cfg.CORES)
        for c in range(cfg.CORES):
            for k, v in in_maps[c].items():
                sim.cores[c].tensor(k)[:] = v
        sim.simulate()
        outs = [sim.cores[c].tensor("outT").copy() for c in range(cfg.CORES)]
        if taps:
            info["taps"] = [
                {k: sim.cores[c].tensor(k).copy() for k in ("tap1", "tap2")}
                for c in range(cfg.CORES)
            ]
    else:
        import concourse.bass_utils as bu

        if trace:
            bu.upload_artifacts = lambda d: "(local)"
        r = bu.run_bass_kernel_spmd(
            nc, in_maps, list(range(cfg.CORES)), trace=trace,
            tmpdir=(inputs.get("_tracedir") if trace else None),
        )
        info["exec_time_ns"] = r.exec_time_ns
        info["mean_exec_time_ns"] = r.mean_exec_time_ns
        outs = [r.results[c]["outT"] for c in range(cfg.CORES)]
        if taps:
            info["taps"] = [
                {k: r.results[c][k] for k in ("tap1", "tap2")} for c in range(cfg.CORES)
            ]

    out = np.concatenate(
        [np.asarray(o, dtype=np.float32)[:, : cfg.NPC].T for o in outs], axis=0
    )
    return out.astype(np.float32), info


def kernel(**inputs):
    out, _ = _run(Cfg(), inputs, mode="hw")
    return out


# revision 3
# speedup vs baseline: 1.2198x; 1.1895x over previous
"""Two-layer GCN (PyG GCNConv x2, relu between) on 8 trn2 NeuronCores. v2.

Strategy (dst-node partitioned, all on-device math):
  - Nodes sharded across 8 cores by destination row (12500/core).
  - Per-layer node tables (dinv * (x@W1), then dinv*relu(h1)) are computed
    shard-wise, AllGather'ed into a replicated bf16 DRAM table of 128B rows,
    and per-edge messages fetched with GPSIMD dma_gather at pair-row (256B)
    granularity (two nodes per gathered row; parity picks the half).
  - Aggregation = PE matmul with one-hot S as the *stationary* operand
    (FWL path): psum[dst,hid] += S[msg,dst]^T @ G[msg,hid].  S built on DVE
    via is_equal against an iota row; 255-sentinel slots contribute nothing,
    letting chunks span dst blocks (segment-packed schedule, less padding).
  - Self-loops are excluded from the edge stream and added with one identity
    matmul per block reading the local stag tile.
  - Epilogues use per-partition dinv columns (tensor_scalar); W2 is applied
    in the layer-2 epilogue via a PE transpose per block.
"""

import math
import sys

sys.path.insert(0, "/opt/trn_rl_repo")

import numpy as np
import ml_dtypes

BF16 = ml_dtypes.bfloat16


class Cfg:
    CORES = 8
    N = 100000
    IN_C = 128
    HID = 64
    OUT_C = 40
    NPC = 12500
    NPC_PAD = 12544  # 98*128
    BLK = 128
    SBB = 6  # dst blocks per superblock (PSUM: 8 banks; SBB + pt + po <= 8)
    GROUP_PAIRS = 32768  # int16 index reach per gather call (pair rows)
    SG = 8  # tasks per S-build op
    MAXCH = 16  # max chunks per dma_gather call (multi-packet)
    NQ = 4  # SWDGE queues (desc-gen parallelism)

    @property
    def NBLK(self):
        return self.NPC_PAD // self.BLK

    @property
    def NSB(self):
        return math.ceil(self.NBLK / self.SBB)

    @property
    def TAB(self):
        return self.NPC_PAD * self.CORES

    @property
    def NGRP(self):
        return math.ceil((self.TAB // 2) / self.GROUP_PAIRS)


# ---------------------------------------------------------------------------
# host-side prep: shard edges, build shared static schedule + per-core arrays
# ---------------------------------------------------------------------------
def _prepare(cfg, edge_index):
    src = np.asarray(edge_index[0], dtype=np.int64)
    dst = np.asarray(edge_index[1], dtype=np.int64)

    deg = np.bincount(dst, minlength=cfg.N).astype(np.float32) + 1.0  # + self loop
    dinv = 1.0 / np.sqrt(deg)

    owner = dst // cfg.NPC
    dl_all = dst - owner * cfg.NPC
    blk_all = dl_all // cfg.BLK
    sb_all = blk_all // cfg.SBB
    srow_all = (src // cfg.NPC) * cfg.NPC_PAD + (src % cfg.NPC)
    pair_all = srow_all >> 1
    par_all = srow_all & 1
    grp_all = pair_all // cfg.GROUP_PAIRS

# per-core sorted edge streams and per-(cell, block) counts
    per_core = []
    counts = np.zeros(
        (cfg.CORES, cfg.NSB, cfg.NGRP, 2, cfg.NBLK), dtype=np.int64
    )
    for c in range(cfg.CORES):
        m = owner == c
        pr, pa, g, b, dl, s = (
            pair_all[m],
            par_all[m],
            grp_all[m],
            blk_all[m],
            dl_all[m],
            sb_all[m],
        )
        key = ((s * cfg.NGRP + g) * 2 + pa) * cfg.NBLK + b
        order = np.argsort(key, kind="stable")
        per_core.append((pr[order], dl[order], key[order]))
        np.add.at(counts[c], (s, g, pa, b), 1)

    segmax = counts.max(axis=0)  # [NSB, NGRP, 2, NBLK] max msgs per segment

    # shared static schedule:
    #   cells iterate (sb, grp, par); within a cell, block segments of length
    #   segmax are packed back to back, cell padded to a chunk multiple.
    calls = []  # (sb, grp, par, first_chunk, nch) per dma_gather call
    tasks = []  # (chunk, block) in emission order
    chunk_tasks = []  # per chunk: list of task ids
    cell_meta = []  # (sb, grp, par, slot0, seg_offsets, nch)
    nslot = 0
    nchunk = 0
    for sb in range(cfg.NSB):
        blo = sb * cfg.SBB
        bhi = min((sb + 1) * cfg.SBB, cfg.NBLK)
        for g in range(cfg.NGRP):
            for pa in range(2):
                segs = segmax[sb, g, pa, blo:bhi]
                tot = int(segs.sum())
                nch = max(1, math.ceil(tot / cfg.BLK))
                offs = np.concatenate([[0], np.cumsum(segs)])
                cell_meta.append((sb, g, pa, nslot, offs, nch))
                # tasks: (chunk, block) for every chunk x overlapping segment
                for j in range(nch):
                    chunk_tasks.append([])
                    s0, s1 = j * cfg.BLK, (j + 1) * cfg.BLK
                    for bi in range(bhi - blo):
                        if offs[bi] < s1 and offs[bi + 1] > s0 and segs[bi] > 0:
                            chunk_tasks[-1].append(len(tasks))
                            tasks.append((nchunk + j, blo + bi))
                # gather calls (split evenly, each <= MAXCH chunks)
                npart = math.ceil(nch / cfg.MAXCH)
                done = 0
                for ki in range(npart):
                    take = (nch - done + npart - ki - 1) // (npart - ki)
                    calls.append((sb, g, pa, nchunk + done, take))
                    done += take
                nslot += nch * cfg.BLK
                nchunk += nch

    ntask = len(tasks)

    # per-core slot arrays (idx + per-task dstloc columns)
    idx_maps = []
    dstloc_maps = []
    for c in range(cfg.CORES):
        pr, dl, key = per_core[c]
        idx_arr = np.zeros(nslot, dtype=np.int64)
        dloc_arr = np.full(nslot, 255, dtype=np.int64)
        dblk_arr = np.full(nslot, -1, dtype=np.int64)
        pos = 0
        ei = 0
        for sb, g, pa, slot0, offs, nch in cell_meta:
            blo = sb * cfg.SBB
            bhi = min((sb + 1) * cfg.SBB, cfg.NBLK)
            for bi in range(bhi - blo):
                n = int(counts[c, sb, g, pa, blo + bi])
                p0 = slot0 + int(offs[bi])
                idx_arr[p0 : p0 + n] = pr[ei : ei + n] - g * cfg.GROUP_PAIRS
                dloc_arr[p0 : p0 + n] = dl[ei : ei + n] % cfg.BLK
                dblk_arr[p0 : p0 + n] = blo + bi
                ei += n
            pos = slot0 + nch * cfg.BLK
        assert ei == len(pr)
        assert idx_arr.min() >= 0 and idx_arr.max() < cfg.GROUP_PAIRS
        idx_maps.append(np.tile(idx_arr.astype(np.int16).reshape(-1, 16).T, (8, 1)))
        # per-task dstloc columns: dl%128 where the slot's block == task block
        dl_mat = dloc_arr.reshape(nchunk, cfg.BLK)
        db_mat = dblk_arr.reshape(nchunk, cfg.BLK)
        dt = np.full((ntask, cfg.BLK), 255, dtype=np.int64)
        for t, (j, b) in enumerate(tasks):
            sel = db_mat[j] == b
            dt[t, sel] = dl_mat[j, sel]
        dstloc_maps.append(np.ascontiguousarray(dt.astype(np.float32).T))  # [128, ntask]

    return {
        "dinv": dinv,
        "calls": calls,
        "tasks": tasks,
        "chunk_tasks": chunk_tasks,
        "cell_meta": cell_meta,
        "nchunk": nchunk,
        "nslot": nslot,
        "ntask": ntask,
        "idx_maps": idx_maps,
        "dstloc_maps": dstloc_maps,
    }


# ---------------------------------------------------------------------------
# device program
# ---------------------------------------------------------------------------
def _build(cfg, prep, debug=False, taps=False):
    import concourse.bacc as bacc
    import concourse.mybir as mybir
    import concourse.tile as tile
    from concourse import library_config

    fp32 = mybir.dt.float32
    bf16 = mybir.dt.bfloat16
    AF = mybir.ActivationFunctionType
    ALU = mybir.AluOpType

    calls = prep["calls"]
    tasks = prep["tasks"]
    chunk_tasks = prep["chunk_tasks"]
    nchunk = prep["nchunk"]
    nslot = prep["nslot"]
    ntask = prep["ntask"]

    # last task index per block (for PSUM stop flags)
    last_task_of_blk = {}
    for t, (j, b) in enumerate(tasks):
        last_task_of_blk[b] = t
    max_call_ch = max(n for _, _, _, _, n in calls)

    nc = bacc.Bacc("TRN2", target_bir_lowering=False, debug=debug, num_swdge_queues=cfg.NQ, dynamic_dma_scratch_size=32768)

    xT_in = nc.dram_tensor("xT", [cfg.IN_C, cfg.NPC_PAD], bf16, kind="ExternalInput")
    W1_in = nc.dram_tensor("W1", [cfg.IN_C, cfg.HID], bf16, kind="ExternalInput")
    W2_in = nc.dram_tensor("W2", [cfg.HID, cfg.OUT_C], bf16, kind="ExternalInput")
    b1r_in = nc.dram_tensor("b1rep", [cfg.BLK, cfg.HID], fp32, kind="ExternalInput")
    b2c_in = nc.dram_tensor("b2col", [cfg.OUT_C, 1], fp32, kind="ExternalInput")
    dinv_in = nc.dram_tensor("dinvcol", [cfg.BLK, cfg.NBLK], fp32, kind="ExternalInput")
    ident_in = nc.dram_tensor("ident", [cfg.BLK, cfg.BLK], bf16, kind="ExternalInput")
    iota_in = nc.dram_tensor("iota", [cfg.BLK, cfg.SG * cfg.BLK], fp32, kind="ExternalInput")
    idx_in = nc.dram_tensor("idxs", [128, nslot // 16], mybir.dt.int16, kind="ExternalInput")
    dstloc_in = nc.dram_tensor("dstloc", [cfg.BLK, ntask], fp32, kind="ExternalInput")
    out_t = nc.dram_tensor("outT", [cfg.OUT_C, cfg.NPC_PAD], bf16, kind="ExternalOutput")
    if taps:
        tap1 = nc.dram_tensor("tap1", [cfg.NPC_PAD, cfg.HID], bf16, kind="ExternalOutput")
        tap2 = nc.dram_tensor("tap2", [cfg.NPC_PAD, cfg.HID], bf16, kind="ExternalOutput")

    shard1 = nc.dram_tensor("shard1", [cfg.NPC_PAD, cfg.HID], bf16)
    shard2 = nc.dram_tensor("shard2", [cfg.NPC_PAD, cfg.HID], bf16)
    table1 = nc.dram_tensor("table1", [cfg.TAB, cfg.HID], bf16, addr_space="Shared")
    table2 = nc.dram_tensor("table2", [cfg.TAB, cfg.HID], bf16, addr_space="Shared")

    replica = [list(range(cfg.CORES))]
    npairs = cfg.TAB // 2

    with tile.TileContext(nc) as tc:
        with (
            tc.tile_pool(name="cst", bufs=1) as cst,
            tc.tile_pool(name="stg", bufs=1) as stg,
            tc.tile_pool(name="gp", bufs=10) as gp,
            tc.tile_pool(name="sp", bufs=10) as sp,
            tc.tile_pool(name="ev", bufs=6) as ev,
        ):
            nc.gpsimd.load_library(library_config.mlp)

            # ---- constants ----
            W1t = cst.tile([cfg.IN_C, cfg.HID], bf16)
            nc.sync.dma_start(W1t[:], W1_in[:])
            W2t = cst.tile([cfg.HID, cfg.OUT_C], bf16)
            nc.sync.dma_start(W2t[:], W2_in[:])
            b1rt = cst.tile([cfg.BLK, cfg.HID], fp32)
            nc.sync.dma_start(b1rt[:], b1r_in[:])
            b2t = cst.tile([cfg.OUT_C, 1], fp32)
            nc.sync.dma_start(b2t[:], b2c_in[:])
            dinvt = cst.tile([cfg.BLK, cfg.NBLK], fp32)
            nc.sync.dma_start(dinvt[:], dinv_in[:])
            identt = cst.tile([cfg.BLK, cfg.BLK], bf16)
            nc.sync.dma_start(identt[:], ident_in[:])
            iota = cst.tile([cfg.BLK, cfg.SG * cfg.BLK], fp32)
            nc.sync.dma_start(iota[:], iota_in[:])
            idxt = cst.tile([128, nslot // 16], mybir.dt.int16)
            nc.sync.dma_start(idxt[:], idx_in[:])
            dstloct = cst.tile([cfg.BLK, ntask], fp32)
            nc.sync.dma_start(dstloct[:], dstloc_in[:])

            stag1 = stg.tile([cfg.BLK, cfg.NBLK, cfg.HID], bf16)
            stag2 = stg.tile([cfg.BLK, cfg.NBLK, cfg.HID], bf16)
            outT = stg.tile([cfg.OUT_C, cfg.NPC_PAD], bf16)

            # ---- layer-1 transform: stag1 = dinv * (x @ W1) ----
            with (
                tc.tile_pool(name="phA", bufs=1) as pa,
                tc.tile_pool(name="psA", bufs=4, space="PSUM") as psA,
            ):
                xtile = pa.tile([cfg.IN_C, cfg.NPC_PAD], bf16)
                nc.sync.dma_start(xtile[:], xT_in[:])
                for b in range(cfg.NBLK):
                    ps = psA.tile([cfg.BLK, cfg.HID], fp32)
                    nc.tensor.matmul(
                        ps[:],
                        lhsT=xtile[:, b * cfg.BLK : (b + 1) * cfg.BLK],
                        rhs=W1t[:],
                        start=True,
                        stop=True,
                    )
                    nc.vector.tensor_tensor(
                        out=stag1[:, b : b + 1, :],
                        in0=ps[:].rearrange("p (a d) -> p a d", a=1),
                        in1=dinvt[:, b : b + 1].to_broadcast([cfg.BLK, 1, cfg.HID]),
                        op=ALU.mult,
                    )
                nc.sync.dma_start(
                    shard1.rearrange("(b p) d -> p b d", p=cfg.BLK)[:], stag1[:]
                )

            nc.gpsimd.collective_compute(
                "AllGather",
                mybir.AluOpType.bypass,
                replica_groups=replica,
                ins=[shard1[:]],
                outs=[table1[:]],
            )
            if taps:
                nc.sync.dma_start(tap1[:], shard1[:])

            # ---- aggregation (shared for both layers) ----
            def agg_layer(layer, table, stag_src, epilogue):
                pairs = table.rearrange("(r two) d -> r (two d)", two=2)
                s_tiles = {}

                def s_for(t):
                    gi = t // cfg.SG
                    if gi not in s_tiles:
                        n = min(cfg.SG, ntask - gi * cfg.SG)
                        st = sp.tile([128, cfg.SG * cfg.BLK], bf16, tag="s")
                        nc.vector.tensor_tensor(
                            out=st[:].rearrange("p (a b) -> p a b", b=cfg.BLK)[:, :n, :],
                            in0=iota[:].rearrange("p (a b) -> p a b", b=cfg.BLK)[
                                :, :n, :
                            ],
                            in1=dstloct[:, gi * cfg.SG : gi * cfg.SG + n].to_broadcast(
                                [128, n, cfg.BLK]
                            ),
                            op=ALU.is_equal,
                        )
                        s_tiles[gi] = st
                    return s_tiles[gi], (t % cfg.SG)

                for sb in range(cfg.NSB):
                    blo = sb * cfg.SBB
                    bhi = min((sb + 1) * cfg.SBB, cfg.NBLK)
                    psums = {}
                    for b in range(blo, bhi):
                        ps = tc_psum.tile([cfg.BLK, cfg.HID], fp32, tag=f"ps{layer}")
                        psums[b] = ps
                        nc.tensor.matmul(
                            ps[:],
                            lhsT=identt[:],
                            rhs=stag_src[:, b, :],
                            start=True,
                            stop=(last_task_of_blk.get(b) is None),
                        )
                    # gather + aggregate this superblock's calls
                    for ci, (call_sb, g, pa_, j0, nch) in enumerate(calls):
                        if call_sb != sb:
                            continue
                        base = g * cfg.GROUP_PAIRS
                        rows = min(cfg.GROUP_PAIRS, npairs - base)
                        gt = gp.tile([128, max_call_ch, 2 * cfg.HID], bf16, tag="g")
                        nc.gpsimd.dma_gather(
                            gt[:, :nch, :],
                            pairs[base : base + rows, :],
                            idxt[:, (j0 * cfg.BLK) // 16 : ((j0 + nch) * cfg.BLK) // 16],
                            nch * cfg.BLK,
                            nch * cfg.BLK,
                            2 * cfg.HID,
                            single_packet=False,
                            queue_num=ci % cfg.NQ,
                        )
                        half = slice(pa_ * cfg.HID, (pa_ + 1) * cfg.HID)
                        for j in range(j0, j0 + nch):
                            for t in chunk_tasks[j]:
                                _, b = tasks[t]
                                st, k = s_for(t)
                                nc.tensor.matmul(
                                    psums[b][:],
                                    lhsT=st[:, k * cfg.BLK : (k + 1) * cfg.BLK],
                                    rhs=gt[:, j - j0, half],
                                    start=False,
                                    stop=(t == last_task_of_blk[b]),
                                )
                    for b in range(blo, bhi):
                        epilogue(b, psums.pop(b))

            # layer 1 aggregation
            def epi1(b, ps):
                u = ev.tile([cfg.BLK, cfg.HID], fp32, tag="u")
                nc.vector.tensor_tensor(
                    out=u[:].rearrange("p (a d) -> p a d", a=1),
                    in0=ps[:].rearrange("p (a d) -> p a d", a=1),
                    in1=dinvt[:, b : b + 1].to_broadcast([cfg.BLK, 1, cfg.HID]),
                    op=ALU.mult,
                )
                v = ev.tile([cfg.BLK, cfg.HID], fp32, tag="v")
                nc.vector.tensor_tensor(out=v[:], in0=u[:], in1=b1rt[:], op=ALU.add)
                w = ev.tile([cfg.BLK, cfg.HID], fp32, tag="w")
                nc.scalar.activation(w[:], v[:], AF.Relu)
                nc.vector.tensor_tensor(
                    out=stag2[:, b : b + 1, :],
                    in0=w[:].rearrange("p (a d) -> p a d", a=1),
                    in1=dinvt[:, b : b + 1].to_broadcast([cfg.BLK, 1, cfg.HID]),
                    op=ALU.mult,
                )

            with tc.tile_pool(name="psagg1", bufs=cfg.SBB, space="PSUM") as tc_psum:
                agg_layer(1, table1, stag1, epi1)
                sh2v = shard2.rearrange("(b p) d -> p b d", p=cfg.BLK)
                for sb in range(cfg.NSB):
                    blo = sb * cfg.SBB
                    bhi = min((sb + 1) * cfg.SBB, cfg.NBLK)
                    nc.sync.dma_start(sh2v[:, blo:bhi, :], stag2[:, blo:bhi, :])
            nc.gpsimd.collective_compute(
                "AllGather",
                mybir.AluOpType.bypass,
                replica_groups=replica,
                ins=[shard2[:]],
                outs=[table2[:]],
            )
            if taps:
                nc.sync.dma_start(tap2[:], shard2[:])

            # layer 2 aggregation -> transposed output
            def epi2(b, ps):
                z = ev.tile([cfg.BLK, cfg.HID], bf16, tag="z")
                nc.vector.tensor_tensor(
                    out=z[:].rearrange("p (a d) -> p a d", a=1),
                    in0=ps[:].rearrange("p (a d) -> p a d", a=1),
                    in1=dinvt[:, b : b + 1].to_broadcast([cfg.BLK, 1, cfg.HID]),
                    op=ALU.mult,
                )
                pst = tc_pt.tile([cfg.HID, cfg.BLK], fp32, tag="pt")
                nc.tensor.matmul(pst[:], lhsT=z[:], rhs=identt[:], start=True, stop=True)
                zt = ev.tile([cfg.HID, cfg.BLK], bf16, tag="zt")
                nc.scalar.activation(zt[:], pst[:], AF.Copy)
                pso = tc_po.tile([cfg.OUT_C, cfg.BLK], fp32, tag="po")
                nc.tensor.matmul(pso[:], lhsT=W2t[:], rhs=zt[:], start=True, stop=True)
                nc.vector.tensor_tensor(
                    out=outT[:, b * cfg.BLK : (b + 1) * cfg.BLK].rearrange(
                        "p (a d) -> p a d", a=1
                    ),
                    in0=pso[:].rearrange("p (a d) -> p a d", a=1),
                    in1=b2t[:, 0:1].to_broadcast([cfg.OUT_C, 1, cfg.BLK]),
                    op=ALU.add,
                )

            with (
                tc.tile_pool(name="psagg2", bufs=cfg.SBB, space="PSUM") as tc_psum,
                tc.tile_pool(name="pspt", bufs=1, space="PSUM") as tc_pt,
                tc.tile_pool(name="pspo", bufs=1, space="PSUM") as tc_po,
            ):
                agg_layer(2, table2, stag2, epi2)

            nc.sync.dma_start(out_t[:], outT[:])

    nc.compile()
    return nc


# ---------------------------------------------------------------------------
# public entry point
# ---------------------------------------------------------------------------
def _make_in_maps(cfg, prep, x, W1, b1, W2, b2):
    dinv = prep["dinv"]
    iota = np.tile(np.arange(cfg.BLK, dtype=np.float32), (cfg.BLK, cfg.SG)).reshape(
        cfg.BLK, cfg.SG * cfg.BLK
    )
    in_maps = []
    for c in range(cfg.CORES):
        xs = np.asarray(x[c * cfg.NPC : (c + 1) * cfg.NPC], np.float32)
        xT = np.zeros((cfg.IN_C, cfg.NPC_PAD), np.float32)
        xT[:, : cfg.NPC] = xs.T
        dv = np.zeros(cfg.NPC_PAD, np.float32)
        dv[: cfg.NPC] = dinv[c * cfg.NPC : (c + 1) * cfg.NPC]
        in_maps.append(
            {
                "xT": xT.astype(BF16),
                "W1": np.asarray(W1, np.float32).astype(BF16),
                "W2": np.asarray(W2, np.float32).astype(BF16),
                "b1rep": np.tile(np.asarray(b1, np.float32), (cfg.BLK, 1)),
                "b2col": np.asarray(b2, np.float32).reshape(cfg.OUT_C, 1),
                "dinvcol": np.ascontiguousarray(
                    dv.reshape(cfg.NBLK, cfg.BLK).T
                ),
                "ident": np.eye(cfg.BLK, dtype=np.float32).astype(BF16),
                "iota": iota.astype(np.float32),
                "idxs": prep["idx_maps"][c],
                "dstloc": prep["dstloc_maps"][c],
            }
        )
    return in_maps


def _run(cfg, inputs, mode="hw", trace=False, taps=False):
    x = np.asarray(inputs["x"], np.float32)
    edge_index = np.asarray(inputs["edge_index"])
    W1 = np.asarray(inputs["W1"], np.float32)
    b1 = np.asarray(inputs["b1"], np.float32)
    W2 = np.asarray(inputs["W2"], np.float32)
    b2 = np.asarray(inputs["b2"], np.float32)

    prep = _prepare(cfg, edge_index)
    nc = _build(cfg, prep, debug=(mode == "sim"), taps=taps)
    in_maps = _make_in_maps(cfg, prep, x, W1, b1, W2, b2)

    info = {}
    if mode == "sim":
        from concourse.bass_interp import MultiCoreSim

        sim = MultiCoreSim(nc, cfg.CORES)
        for c in range(cfg.CORES):
            for k, v in in_maps[c].items():
                sim.cores[c].tensor(k)[:] = v
        sim.simulate()
        outs = [sim.cores[c].tensor("outT").copy() for c in range(cfg.CORES)]
        if taps:
            info["taps"] = [
                {k: sim.cores[c].tensor(k).copy() for k in ("tap1", "tap2")}
                for c in range(cfg.CORES)
            ]
    else:
        import concourse.bass_utils as bu

        if trace:
            bu.upload_artifacts = lambda d: "(local)"
        r = bu.run_bass_kernel_spmd(
            nc, in_maps, list(range(cfg.CORES)), trace=trace,
            tmpdir=(inputs.get("_tracedir") if trace else None),
        )
        info["exec_time_ns"] = r.exec_time_ns
        info["mean_exec_time_ns"] = r.mean_exec_time_ns
        outs = [r.results[c]["outT"] for c in range(cfg.CORES)]
        if taps:
            info["taps"] = [
                {k: r.results[c][k] for k in ("tap1", "tap2")} for c in range(cfg.CORES)
            ]

    out = np.concatenate(
        [np.asarray(o, dtype=np.float32)[:, : cfg.NPC].T for o in outs], axis=0
    )
    return out.astype(np.float32), info


def kernel(**inputs):
    out, _ = _run(Cfg(), inputs, mode="hw")
    return out


# revision 4
# speedup vs baseline: 1.2204x; 1.0005x over previous
"""Two-layer GCN (PyG GCNConv x2, relu between) on 8 trn2 NeuronCores. v2.

Strategy (dst-node partitioned, all on-device math):
  - Nodes sharded across 8 cores by destination row (12500/core).
  - Per-layer node tables (dinv * (x@W1), then dinv*relu(h1)) are computed
    shard-wise, AllGather'ed into a replicated bf16 DRAM table of 128B rows,
    and per-edge messages fetched with GPSIMD dma_gather at pair-row (256B)
    granularity (two nodes per gathered row; parity picks the half).
  - Aggregation = PE matmul with one-hot S as the *stationary* operand
    (FWL path): psum[dst,hid] += S[msg,dst]^T @ G[msg,hid].  S built on DVE
    via is_equal against an iota row; 255-sentinel slots contribute nothing,
    letting chunks span dst blocks (segment-packed schedule, less padding).
  - Self-loops are excluded from the edge stream and added with one identity
    matmul per block reading the local stag tile.
  - Epilogues use per-partition dinv columns (tensor_scalar); W2 is applied
    in the layer-2 epilogue via a PE transpose per block.
"""

import math
import sys

sys.path.insert(0, "/opt/trn_rl_repo")

import numpy as np
import ml_dtypes

BF16 = ml_dtypes.bfloat16


class Cfg:
    CORES = 8
    N = 100000
    IN_C = 128
    HID = 64
    OUT_C = 40
    NPC = 12500
    NPC_PAD = 12544  # 98*128
    BLK = 128
    SBB = 6  # dst blocks per superblock (PSUM: 8 banks; SBB + pt + po <= 8)
    GROUP_PAIRS = 32768  # int16 index reach per gather call (pair rows)
    SG = 8  # tasks per S-build op
    MAXCH = 16  # max chunks per dma_gather call (multi-packet)
    NQ = 4  # SWDGE queues (desc-gen parallelism)

    @property
    def NBLK(self):
        return self.NPC_PAD // self.BLK

    @property
    def NSB(self):
        return math.ceil(self.NBLK / self.SBB)

    @property
    def TAB(self):
        return self.NPC_PAD * self.CORES

    @property
    def NGRP(self):
        return math.ceil((self.TAB // 2) / self.GROUP_PAIRS)

    @property
    def HALF(self):
        return self.NPC_PAD // 2

    @property
    def NBH(self):
        return self.HALF // self.BLK

    @property
    def TABH(self):
        return self.HALF * self.CORES


# ---------------------------------------------------------------------------
# host-side prep: shard edges, build shared static schedule + per-core arrays
# ---------------------------------------------------------------------------
def _prepare(cfg, edge_index):
    src = np.asarray(edge_index[0], dtype=np.int64)
    dst = np.asarray(edge_index[1], dtype=np.int64)

    deg = np.bincount(dst, minlength=cfg.N).astype(np.float32) + 1.0  # + self loop
    dinv = 1.0 / np.sqrt(deg)

    owner = dst // cfg.NPC
    dl_all = dst - owner * cfg.NPC
    blk_all = dl_all // cfg.BLK
    sb_all = blk_all // cfg.SBB
    r_all = src % cfg.NPC
    half_all = r_all // cfg.HALF
    rowh_all = (src // cfg.NPC) * cfg.HALF + (r_all - half_all * cfg.HALF)
    pair_all = rowh_all >> 1
    par_all = rowh_all & 1
    grp_all = half_all  # "group" = table half

# per-core sorted edge streams and per-(cell, block) counts
    per_core = []
    counts = np.zeros(
        (cfg.CORES, cfg.NSB, cfg.NGRP, 2, cfg.NBLK), dtype=np.int64
    )
    for c in range(cfg.CORES):
        m = owner == c
        pr, pa, g, b, dl, s = (
            pair_all[m],
            par_all[m],
            grp_all[m],
            blk_all[m],
            dl_all[m],
            sb_all[m],
        )
        key = ((s * cfg.NGRP + g) * 2 + pa) * cfg.NBLK + b
        order = np.argsort(key, kind="stable")
        per_core.append((pr[order], dl[order], key[order]))
        np.add.at(counts[c], (s, g, pa, b), 1)

    segmax = counts.max(axis=0)  # [NSB, NGRP, 2, NBLK] max msgs per segment

    # shared static schedule:
    #   cells iterate (sb, grp, par); within a cell, block segments of length
    #   segmax are packed back to back, cell padded to a chunk multiple.
    calls = []  # (sb, grp, par, first_chunk, nch) per dma_gather call
    tasks = []  # (chunk, block) in emission order
    chunk_tasks = []  # per chunk: list of task ids
    cell_meta = []  # (sb, grp, par, slot0, seg_offsets, nch)
    nslot = 0
    nchunk = 0
    for sb in range(cfg.NSB):
        blo = sb * cfg.SBB
        bhi = min((sb + 1) * cfg.SBB, cfg.NBLK)
        for g in range(cfg.NGRP):
            for pa in range(2):
                segs = segmax[sb, g, pa, blo:bhi]
                tot = int(segs.sum())
                nch = max(1, math.ceil(tot / cfg.BLK))
                offs = np.concatenate([[0], np.cumsum(segs)])
                cell_meta.append((sb, g, pa, nslot, offs, nch))
                # tasks: (chunk, block) for every chunk x overlapping segment
                for j in range(nch):
                    chunk_tasks.append([])
                    s0, s1 = j * cfg.BLK, (j + 1) * cfg.BLK
                    for bi in range(bhi - blo):
                        if offs[bi] < s1 and offs[bi + 1] > s0 and segs[bi] > 0:
                            chunk_tasks[-1].append(len(tasks))
                            tasks.append((nchunk + j, blo + bi))
                # gather calls (split evenly, each <= MAXCH chunks)
                npart = math.ceil(nch / cfg.MAXCH)
                done = 0
                for ki in range(npart):
                    take = (nch - done + npart - ki - 1) // (npart - ki)
                    calls.append((sb, g, pa, nchunk + done, take))
                    done += take
                nslot += nch * cfg.BLK
                nchunk += nch

    ntask = len(tasks)

    # per-core slot arrays (idx + per-task dstloc columns)
    idx_maps = []
    dstloc_maps = []
    for c in range(cfg.CORES):
        pr, dl, key = per_core[c]
        idx_arr = np.zeros(nslot, dtype=np.int64)
        dloc_arr = np.full(nslot, 255, dtype=np.int64)
        dblk_arr = np.full(nslot, -1, dtype=np.int64)
        pos = 0
        ei = 0
        for sb, g, pa, slot0, offs, nch in cell_meta:
            blo = sb * cfg.SBB
            bhi = min((sb + 1) * cfg.SBB, cfg.NBLK)
            for bi in range(bhi - blo):
                n = int(counts[c, sb, g, pa, blo + bi])
                p0 = slot0 + int(offs[bi])
                idx_arr[p0 : p0 + n] = pr[ei : ei + n]
                dloc_arr[p0 : p0 + n] = dl[ei : ei + n] % cfg.BLK
                dblk_arr[p0 : p0 + n] = blo + bi
                ei += n
            pos = slot0 + nch * cfg.BLK
        assert ei == len(pr)
        assert idx_arr.min() >= 0 and idx_arr.max() < min(32768, cfg.TABH // 2)
        idx_maps.append(np.tile(idx_arr.astype(np.int16).reshape(-1, 16).T, (8, 1)))
        # per-task dstloc columns: dl%128 where the slot's block == task block
        dl_mat = dloc_arr.reshape(nchunk, cfg.BLK)
        db_mat = dblk_arr.reshape(nchunk, cfg.BLK)
        dt = np.full((ntask, cfg.BLK), 255, dtype=np.int64)
        for t, (j, b) in enumerate(tasks):
            sel = db_mat[j] == b
            dt[t, sel] = dl_mat[j, sel]
        dstloc_maps.append(np.ascontiguousarray(dt.astype(np.float32).T))  # [128, ntask]

    return {
        "dinv": dinv,
        "calls": calls,
        "tasks": tasks,
        "chunk_tasks": chunk_tasks,
        "cell_meta": cell_meta,
        "nchunk": nchunk,
        "nslot": nslot,
        "ntask": ntask,
        "idx_maps": idx_maps,
        "dstloc_maps": dstloc_maps,
    }


# ---------------------------------------------------------------------------
# device program
# ---------------------------------------------------------------------------
def _build(cfg, prep, debug=False, taps=False):
    import concourse.bacc as bacc
    import concourse.mybir as mybir
    import concourse.tile as tile
    from concourse import library_config

    fp32 = mybir.dt.float32
    bf16 = mybir.dt.bfloat16
    AF = mybir.ActivationFunctionType
    ALU = mybir.AluOpType

    calls = prep["calls"]
    tasks = prep["tasks"]
    chunk_tasks = prep["chunk_tasks"]
    nchunk = prep["nchunk"]
    nslot = prep["nslot"]
    ntask = prep["ntask"]

    # last task index per block (for PSUM stop flags)
    last_task_of_blk = {}
    for t, (j, b) in enumerate(tasks):
        last_task_of_blk[b] = t
    max_call_ch = max(n for _, _, _, _, n in calls)

    nc = bacc.Bacc("TRN2", target_bir_lowering=False, debug=debug, num_swdge_queues=cfg.NQ, dynamic_dma_scratch_size=32768)

    xT_in = nc.dram_tensor("xT", [cfg.IN_C, cfg.NPC_PAD], bf16, kind="ExternalInput")
    W1_in = nc.dram_tensor("W1", [cfg.IN_C, cfg.HID], bf16, kind="ExternalInput")
    W2_in = nc.dram_tensor("W2", [cfg.HID, cfg.OUT_C], bf16, kind="ExternalInput")
    b1r_in = nc.dram_tensor("b1rep", [cfg.BLK, cfg.HID], fp32, kind="ExternalInput")
    b2c_in = nc.dram_tensor("b2col", [cfg.OUT_C, 1], fp32, kind="ExternalInput")
    dinv_in = nc.dram_tensor("dinvcol", [cfg.BLK, cfg.NBLK], fp32, kind="ExternalInput")
    ident_in = nc.dram_tensor("ident", [cfg.BLK, cfg.BLK], bf16, kind="ExternalInput")
    iota_in = nc.dram_tensor("iota", [cfg.BLK, cfg.SG * cfg.BLK], fp32, kind="ExternalInput")
    idx_in = nc.dram_tensor("idxs", [128, nslot // 16], mybir.dt.int16, kind="ExternalInput")
    dstloc_in = nc.dram_tensor("dstloc", [cfg.BLK, ntask], fp32, kind="ExternalInput")
    out_t = nc.dram_tensor("outT", [cfg.OUT_C, cfg.NPC_PAD], bf16, kind="ExternalOutput")

    shard1h = [nc.dram_tensor(f"shard1{h}", [cfg.HALF, cfg.HID], bf16) for h in range(2)]
    shard2h = [nc.dram_tensor(f"shard2{h}", [cfg.HALF, cfg.HID], bf16) for h in range(2)]
    table1h = [
        nc.dram_tensor(f"table1{h}", [cfg.TABH, cfg.HID], bf16, addr_space="Shared")
        for h in range(2)
    ]
    table2h = [
        nc.dram_tensor(f"table2{h}", [cfg.TABH, cfg.HID], bf16, addr_space="Shared")
        for h in range(2)
    ]

    replica = [list(range(cfg.CORES))]
    npairs_h = cfg.TABH // 2

    with tile.TileContext(nc) as tc:
        with (
            tc.tile_pool(name="cst", bufs=1) as cst,
            tc.tile_pool(name="stg", bufs=1) as stg,
            tc.tile_pool(name="gp", bufs=10) as gp,
            tc.tile_pool(name="sp", bufs=10) as sp,
            tc.tile_pool(name="ev", bufs=6) as ev,
        ):
            nc.gpsimd.load_library(library_config.mlp)

            # ---- constants ----
            W1t = cst.tile([cfg.IN_C, cfg.HID], bf16)
            nc.sync.dma_start(W1t[:], W1_in[:])
            W2t = cst.tile([cfg.HID, cfg.OUT_C], bf16)
            nc.sync.dma_start(W2t[:], W2_in[:])
            b1rt = cst.tile([cfg.BLK, cfg.HID], fp32)
            nc.sync.dma_start(b1rt[:], b1r_in[:])
            b2t = cst.tile([cfg.OUT_C, 1], fp32)
            nc.sync.dma_start(b2t[:], b2c_in[:])
            dinvt = cst.tile([cfg.BLK, cfg.NBLK], fp32)
            nc.sync.dma_start(dinvt[:], dinv_in[:])
            identt = cst.tile([cfg.BLK, cfg.BLK], bf16)
            nc.sync.dma_start(identt[:], ident_in[:])
            iota = cst.tile([cfg.BLK, cfg.SG * cfg.BLK], fp32)
            nc.sync.dma_start(iota[:], iota_in[:])
            idxt = cst.tile([128, nslot // 16], mybir.dt.int16)
            nc.sync.dma_start(idxt[:], idx_in[:])
            dstloct = cst.tile([cfg.BLK, ntask], fp32)
            nc.sync.dma_start(dstloct[:], dstloc_in[:])

            stag1a = stg.tile([cfg.BLK, cfg.NBH, cfg.HID], bf16)
            stag1b = stg.tile([cfg.BLK, cfg.NBH, cfg.HID], bf16)
            stag2a = stg.tile([cfg.BLK, cfg.NBH, cfg.HID], bf16)
            stag2b = stg.tile([cfg.BLK, cfg.NBH, cfg.HID], bf16)
            stag1h = [stag1a, stag1b]
            stag2h = [stag2a, stag2b]
            outT = stg.tile([cfg.OUT_C, cfg.NPC_PAD], bf16)

            def stag_slice(tiles, b):
                h, bl = (0, b) if b < cfg.NBH else (1, b - cfg.NBH)
                return tiles[h][:, bl : bl + 1, :]

            # ---- layer-1 transform: stag1 = dinv * (x @ W1) ----
            with (
                tc.tile_pool(name="phA", bufs=1) as pa,
                tc.tile_pool(name="psA", bufs=4, space="PSUM") as psA,
            ):
                xtile = pa.tile([cfg.IN_C, cfg.NPC_PAD], bf16)
                nc.sync.dma_start(xtile[:], xT_in[:])
                for b in range(cfg.NBLK):
                    ps = psA.tile([cfg.BLK, cfg.HID], fp32)
                    nc.tensor.matmul(
                        ps[:],
                        lhsT=xtile[:, b * cfg.BLK : (b + 1) * cfg.BLK],
                        rhs=W1t[:],
                        start=True,
                        stop=True,
                    )
                    nc.vector.tensor_tensor(
                        out=stag_slice(stag1h, b),
                        in0=ps[:].rearrange("p (a d) -> p a d", a=1),
                        in1=dinvt[:, b : b + 1].to_broadcast([cfg.BLK, 1, cfg.HID]),
                        op=ALU.mult,
                    )
                for h in range(2):
                    nc.sync.dma_start(
                        shard1h[h].rearrange("(b p) d -> p b d", p=cfg.BLK)[:],
                        stag1h[h][:],
                    )
                    nc.gpsimd.collective_compute(
                        "AllGather",
                        mybir.AluOpType.bypass,
                        replica_groups=replica,
                        ins=[shard1h[h][:]],
                        outs=[table1h[h][:]],
                    )

            # ---- aggregation (shared for both layers) ----
            def agg_layer(layer, tables, stag_tiles, epilogue):
                pairs_h = [t.rearrange("(r two) d -> r (two d)", two=2) for t in tables]
                s_tiles = {}

                def s_for(t):
                    gi = t // cfg.SG
                    if gi not in s_tiles:
                        n = min(cfg.SG, ntask - gi * cfg.SG)
                        st = sp.tile([128, cfg.SG * cfg.BLK], bf16, tag="s")
                        nc.vector.tensor_tensor(
                            out=st[:].rearrange("p (a b) -> p a b", b=cfg.BLK)[:, :n, :],
                            in0=iota[:].rearrange("p (a b) -> p a b", b=cfg.BLK)[
                                :, :n, :
                            ],
                            in1=dstloct[:, gi * cfg.SG : gi * cfg.SG + n].to_broadcast(
                                [128, n, cfg.BLK]
                            ),
                            op=ALU.is_equal,
                        )
                        s_tiles[gi] = st
                    return s_tiles[gi], (t % cfg.SG)

                for sb in range(cfg.NSB):
                    blo = sb * cfg.SBB
                    bhi = min((sb + 1) * cfg.SBB, cfg.NBLK)
                    psums = {}
                    for b in range(blo, bhi):
                        ps = tc_psum.tile([cfg.BLK, cfg.HID], fp32, tag=f"ps{layer}")
                        psums[b] = ps
                        nc.tensor.matmul(
                            ps[:],
                            lhsT=identt[:],
                            rhs=stag_slice(stag_tiles, b).squeeze(1),
                            start=True,
                            stop=(last_task_of_blk.get(b) is None),
                        )
                    # gather + aggregate this superblock's calls
                    for ci, (call_sb, g, pa_, j0, nch) in enumerate(calls):
                        if call_sb != sb:
                            continue
                        gt = gp.tile([128, max_call_ch, 2 * cfg.HID], bf16, tag="g")
                        nc.gpsimd.dma_gather(
                            gt[:, :nch, :],
                            pairs_h[g][0:npairs_h, :],
                            idxt[:, (j0 * cfg.BLK) // 16 : ((j0 + nch) * cfg.BLK) // 16],
                            nch * cfg.BLK,
                            nch * cfg.BLK,
                            2 * cfg.HID,
                            single_packet=False,
                            queue_num=ci % cfg.NQ,
                        )
                        half = slice(pa_ * cfg.HID, (pa_ + 1) * cfg.HID)
                        for j in range(j0, j0 + nch):
                            for t in chunk_tasks[j]:
                                _, b = tasks[t]
                                st, k = s_for(t)
                                nc.tensor.matmul(
                                    psums[b][:],
                                    lhsT=st[:, k * cfg.BLK : (k + 1) * cfg.BLK],
                                    rhs=gt[:, j - j0, half],
                                    start=False,
                                    stop=(t == last_task_of_blk[b]),
                                )
                    for b in range(blo, bhi):
                        epilogue(b, psums.pop(b))

            # layer 1 aggregation
            def epi1(b, ps):
                u = ev.tile([cfg.BLK, cfg.HID], fp32, tag="u")
                nc.vector.tensor_tensor(
                    out=u[:].rearrange("p (a d) -> p a d", a=1),
                    in0=ps[:].rearrange("p (a d) -> p a d", a=1),
                    in1=dinvt[:, b : b + 1].to_broadcast([cfg.BLK, 1, cfg.HID]),
                    op=ALU.mult,
                )
                v = ev.tile([cfg.BLK, cfg.HID], fp32, tag="v")
                nc.vector.tensor_tensor(out=v[:], in0=u[:], in1=b1rt[:], op=ALU.add)
                w = ev.tile([cfg.BLK, cfg.HID], fp32, tag="w")
                nc.scalar.activation(w[:], v[:], AF.Relu)
                nc.vector.tensor_tensor(
                    out=stag_slice(stag2h, b),
                    in0=w[:].rearrange("p (a d) -> p a d", a=1),
                    in1=dinvt[:, b : b + 1].to_broadcast([cfg.BLK, 1, cfg.HID]),
                    op=ALU.mult,
                )

            with tc.tile_pool(name="psagg1", bufs=cfg.SBB, space="PSUM") as tc_psum:
                agg_layer(1, table1h, stag1h, epi1)
                for h in range(2):
                    nc.sync.dma_start(
                        shard2h[h].rearrange("(b p) d -> p b d", p=cfg.BLK)[:],
                        stag2h[h][:],
                    )
                    nc.gpsimd.collective_compute(
                        "AllGather",
                        mybir.AluOpType.bypass,
                        replica_groups=replica,
                        ins=[shard2h[h][:]],
                        outs=[table2h[h][:]],
                    )

            # layer 2 aggregation -> transposed output
            def epi2(b, ps):
                z = ev.tile([cfg.BLK, cfg.HID], bf16, tag="z")
                nc.vector.tensor_tensor(
                    out=z[:].rearrange("p (a d) -> p a d", a=1),
                    in0=ps[:].rearrange("p (a d) -> p a d", a=1),
                    in1=dinvt[:, b : b + 1].to_broadcast([cfg.BLK, 1, cfg.HID]),
                    op=ALU.mult,
                )
                pst = tc_pt.tile([cfg.HID, cfg.BLK], fp32, tag="pt")
                nc.tensor.matmul(pst[:], lhsT=z[:], rhs=identt[:], start=True, stop=True)
                zt = ev.tile([cfg.HID, cfg.BLK], bf16, tag="zt")
                nc.scalar.activation(zt[:], pst[:], AF.Copy)
                pso = tc_po.tile([cfg.OUT_C, cfg.BLK], fp32, tag="po")
                nc.tensor.matmul(pso[:], lhsT=W2t[:], rhs=zt[:], start=True, stop=True)
                nc.vector.tensor_tensor(
                    out=outT[:, b * cfg.BLK : (b + 1) * cfg.BLK].rearrange(
                        "p (a d) -> p a d", a=1
                    ),
                    in0=pso[:].rearrange("p (a d) -> p a d", a=1),
                    in1=b2t[:, 0:1].to_broadcast([cfg.OUT_C, 1, cfg.BLK]),
                    op=ALU.add,
                )

            with (
                tc.tile_pool(name="psagg2", bufs=cfg.SBB, space="PSUM") as tc_psum,
                tc.tile_pool(name="pspt", bufs=1, space="PSUM") as tc_pt,
                tc.tile_pool(name="pspo", bufs=1, space="PSUM") as tc_po,
            ):
                agg_layer(2, table2h, stag2h, epi2)

            nc.sync.dma_start(out_t[:], outT[:])

    nc.compile()
    return nc


# ---------------------------------------------------------------------------
# public entry point
# ---------------------------------------------------------------------------
def _make_in_maps(cfg, prep, x, W1, b1, W2, b2):
    dinv = prep["dinv"]
    iota = np.tile(np.arange(cfg.BLK, dtype=np.float32), (cfg.BLK, cfg.SG)).reshape(
        cfg.BLK, cfg.SG * cfg.BLK
    )
    in_maps = []
    for c in range(cfg.CORES):
        xs = np.asarray(x[c * cfg.NPC : (c + 1) * cfg.NPC], np.float32)
        xT = np.zeros((cfg.IN_C, cfg.NPC_PAD), np.float32)
        xT[:, : cfg.NPC] = xs.T
        dv = np.zeros(cfg.NPC_PAD, np.float32)
        dv[: cfg.NPC] = dinv[c * cfg.NPC : (c + 1) * cfg.NPC]
        in_maps.append(
            {
                "xT": xT.astype(BF16),
                "W1": np.asarray(W1, np.float32).astype(BF16),
                "W2": np.asarray(W2, np.float32).astype(BF16),
                "b1rep": np.tile(np.asarray(b1, np.float32), (cfg.BLK, 1)),
                "b2col": np.asarray(b2, np.float32).reshape(cfg.OUT_C, 1),
                "dinvcol": np.ascontiguousarray(
                    dv.reshape(cfg.NBLK, cfg.BLK).T
                ),
                "ident": np.eye(cfg.BLK, dtype=np.float32).astype(BF16),
                "iota": iota.astype(np.float32),
                "idxs": prep["idx_maps"][c],
                "dstloc": prep["dstloc_maps"][c],
            }
        )
    return in_maps


def _run(cfg, inputs, mode="hw", trace=False, taps=False):
    x = np.asarray(inputs["x"], np.float32)
    edge_index = np.asarray(inputs["edge_index"])
    W1 = np.asarray(inputs["W1"], np.float32)
    b1 = np.asarray(inputs["b1"], np.float32)
    W2 = np.asarray(inputs["W2"], np.float32)
    b2 = np.asarray(inputs["b2"], np.float32)

    prep = _prepare(cfg, edge_index)
    nc = _build(cfg, prep, debug=(mode == "sim"), taps=taps)
    in_maps = _make_in_maps(cfg, prep, x, W1, b1, W2, b2)

    info = {}
    if mode == "sim":
        from concourse.bass_interp import MultiCoreSim

        sim = MultiCoreSim(nc, cfg.CORES)
        for c in range(cfg.CORES):
            for k, v in in_maps[c].items():
                sim.cores[c].tensor(k)[:] = v
        sim.simulate()
        outs = [sim.cores[c].tensor("outT").copy() for c in range(cfg.CORES)]
        if taps:
            info["taps"] = [
                {k: sim.cores[c].tensor(k).copy() for k in ("tap1", "tap2")}
                for c in range(cfg.CORES)
            ]
    else:
        import concourse.bass_utils as bu

        if trace:
            bu.upload_artifacts = lambda d: "(local)"
        r = bu.run_bass_kernel_spmd(
            nc, in_maps, list(range(cfg.CORES)), trace=trace,
            tmpdir=(inputs.get("_tracedir") if trace else None),
        )
        info["exec_time_ns"] = r.exec_time_ns
        info["mean_exec_time_ns"] = r.mean_exec_time_ns
        outs = [r.results[c]["outT"] for c in range(cfg.CORES)]
        if taps:
            info["taps"] = [
                {k: r.results[c][k] for k in ("tap1", "tap2")} for c in range(cfg.CORES)
            ]

    out = np.concatenate(
        [np.asarray(o, dtype=np.float32)[:, : cfg.NPC].T for o in outs], axis=0
    )
    return out.astype(np.float32), info


def kernel(**inputs):
    out, _ = _run(Cfg(), inputs, mode="hw")
    return out


# revision 5
# speedup vs baseline: 1.2528x; 1.0265x over previous
"""Two-layer GCN (PyG GCNConv x2, relu between) on 8 trn2 NeuronCores. v2.

Strategy (dst-node partitioned, all on-device math):
  - Nodes sharded across 8 cores by destination row (12500/core).
  - Per-layer node tables (dinv * (x@W1), then dinv*relu(h1)) are computed
    shard-wise, AllGather'ed into a replicated bf16 DRAM table of 128B rows,
    and per-edge messages fetched with GPSIMD dma_gather at pair-row (256B)
    granularity (two nodes per gathered row; parity picks the half).
  - Aggregation = PE matmul with one-hot S as the *stationary* operand
    (FWL path): psum[dst,hid] += S[msg,dst]^T @ G[msg,hid].  S built on DVE
    via is_equal against an iota row; 255-sentinel slots contribute nothing,
    letting chunks span dst blocks (segment-packed schedule, less padding).
  - Self-loops are excluded from the edge stream and added with one identity
    matmul per block reading the local stag tile.
  - Epilogues use per-partition dinv columns (tensor_scalar); W2 is applied
    in the layer-2 epilogue via a PE transpose per block.
"""

import math
import sys

sys.path.insert(0, "/opt/trn_rl_repo")

import numpy as np
import ml_dtypes

BF16 = ml_dtypes.bfloat16


class Cfg:
    CORES = 8
    N = 100000
    IN_C = 128
    HID = 64
    OUT_C = 40
    NPC = 12500
    NPC_PAD = 12544  # 98*128
    BLK = 128
    SBB = 6  # dst blocks per superblock (PSUM: 8 banks; SBB + pt + po <= 8)
    GROUP_PAIRS = 32768  # int16 index reach per gather call (pair rows)
    SG = 8  # tasks per S-build op
    MAXCH = 16  # max chunks per dma_gather call (multi-packet)
    NQ = 4  # SWDGE queues (desc-gen parallelism)

    @property
    def NBLK(self):
        return self.NPC_PAD // self.BLK

    @property
    def NSB(self):
        return math.ceil(self.NBLK / self.SBB)

    @property
    def TAB(self):
        return self.NPC_PAD * self.CORES

    @property
    def NGRP(self):
        return math.ceil((self.TAB // 2) / self.GROUP_PAIRS)

    @property
    def HALF(self):
        return self.NPC_PAD // 2

    @property
    def NBH(self):
        return self.HALF // self.BLK

    @property
    def TABH(self):
        return self.HALF * self.CORES


# ---------------------------------------------------------------------------
# host-side prep: shard edges, build shared static schedule + per-core arrays
# ---------------------------------------------------------------------------
def _prepare(cfg, edge_index):
    src = np.asarray(edge_index[0], dtype=np.int64)
    dst = np.asarray(edge_index[1], dtype=np.int64)

    deg = np.bincount(dst, minlength=cfg.N).astype(np.float32) + 1.0  # + self loop
    dinv = 1.0 / np.sqrt(deg)

    owner = dst // cfg.NPC
    dl_all = dst - owner * cfg.NPC
    blk_all = dl_all // cfg.BLK
    sb_all = blk_all // cfg.SBB
    r_all = src % cfg.NPC
    half_all = r_all // cfg.HALF
    rowh_all = (src // cfg.NPC) * cfg.HALF + (r_all - half_all * cfg.HALF)
    pair_all = rowh_all >> 1
    par_all = rowh_all & 1
    grp_all = half_all  # "group" = table half

# per-core sorted edge streams and per-(cell, block) counts
    per_core = []
    counts = np.zeros(
        (cfg.CORES, cfg.NSB, cfg.NGRP, 2, cfg.NBLK), dtype=np.int64
    )
    for c in range(cfg.CORES):
        m = owner == c
        pr, pa, g, b, dl, s = (
            pair_all[m],
            par_all[m],
            grp_all[m],
            blk_all[m],
            dl_all[m],
            sb_all[m],
        )
        key = ((s * cfg.NGRP + g) * 2 + pa) * cfg.NBLK + b
        order = np.argsort(key, kind="stable")
        per_core.append((pr[order], dl[order], key[order]))
        np.add.at(counts[c], (s, g, pa, b), 1)

    segmax = counts.max(axis=0)  # [NSB, NGRP, 2, NBLK] max msgs per segment

    # shared static schedule:
    #   cells iterate (sb, grp, par); within a cell, block segments of length
    #   segmax are packed back to back, cell padded to a chunk multiple.
    calls = []  # (sb, grp, par, first_chunk, nch) per dma_gather call
    tasks = []  # (chunk, block) in emission order
    chunk_tasks = []  # per chunk: list of task ids
    cell_meta = []  # (sb, grp, par, slot0, seg_offsets, nch)
    nslot = 0
    nchunk = 0
    for sb in range(cfg.NSB):
        blo = sb * cfg.SBB
        bhi = min((sb + 1) * cfg.SBB, cfg.NBLK)
        for g in range(cfg.NGRP):
            for pa in range(2):
                segs = segmax[sb, g, pa, blo:bhi]
                tot = int(segs.sum())
                nch = max(1, math.ceil(tot / cfg.BLK))
                offs = np.concatenate([[0], np.cumsum(segs)])
                cell_meta.append((sb, g, pa, nslot, offs, nch))
                # tasks: (chunk, block) for every chunk x overlapping segment
                for j in range(nch):
                    chunk_tasks.append([])
                    s0, s1 = j * cfg.BLK, (j + 1) * cfg.BLK
                    for bi in range(bhi - blo):
                        if offs[bi] < s1 and offs[bi + 1] > s0 and segs[bi] > 0:
                            chunk_tasks[-1].append(len(tasks))
                            tasks.append((nchunk + j, blo + bi))
                # gather calls (split evenly, each <= MAXCH chunks)
                npart = math.ceil(nch / cfg.MAXCH)
                done = 0
                for ki in range(npart):
                    take = (nch - done + npart - ki - 1) // (npart - ki)
                    calls.append((sb, g, pa, nchunk + done, take))
                    done += take
                nslot += nch * cfg.BLK
                nchunk += nch

    ntask = len(tasks)

    # per-core slot arrays (idx + per-task dstloc columns)
    idx_maps = []
    dstloc_maps = []
    for c in range(cfg.CORES):
        pr, dl, key = per_core[c]
        idx_arr = np.zeros(nslot, dtype=np.int64)
        dloc_arr = np.full(nslot, 255, dtype=np.int64)
        dblk_arr = np.full(nslot, -1, dtype=np.int64)
        pos = 0
        ei = 0
        for sb, g, pa, slot0, offs, nch in cell_meta:
            blo = sb * cfg.SBB
            bhi = min((sb + 1) * cfg.SBB, cfg.NBLK)
            for bi in range(bhi - blo):
                n = int(counts[c, sb, g, pa, blo + bi])
                p0 = slot0 + int(offs[bi])
                idx_arr[p0 : p0 + n] = pr[ei : ei + n]
                dloc_arr[p0 : p0 + n] = dl[ei : ei + n] % cfg.BLK
                dblk_arr[p0 : p0 + n] = blo + bi
                ei += n
            pos = slot0 + nch * cfg.BLK
        assert ei == len(pr)
        assert idx_arr.min() >= 0 and idx_arr.max() < min(32768, cfg.TABH // 2)
        idx_maps.append(np.tile(idx_arr.astype(np.int16).reshape(-1, 16).T, (8, 1)))
        # per-task dstloc columns: dl%128 where the slot's block == task block
        dl_mat = dloc_arr.reshape(nchunk, cfg.BLK)
        db_mat = dblk_arr.reshape(nchunk, cfg.BLK)
        dt = np.full((ntask, cfg.BLK), 255, dtype=np.int64)
        for t, (j, b) in enumerate(tasks):
            sel = db_mat[j] == b
            dt[t, sel] = dl_mat[j, sel]
        dstloc_maps.append(np.ascontiguousarray(dt.astype(np.float32).T))  # [128, ntask]

    return {
        "dinv": dinv,
        "calls": calls,
        "tasks": tasks,
        "chunk_tasks": chunk_tasks,
        "cell_meta": cell_meta,
        "nchunk": nchunk,
        "nslot": nslot,
        "ntask": ntask,
        "idx_maps": idx_maps,
        "dstloc_maps": dstloc_maps,
    }


# ---------------------------------------------------------------------------
# device program
# ---------------------------------------------------------------------------
def _build(cfg, prep, debug=False, taps=False):
    import concourse.bacc as bacc
    import concourse.mybir as mybir
    import concourse.tile as tile
    from concourse import library_config

    fp32 = mybir.dt.float32
    bf16 = mybir.dt.bfloat16
    AF = mybir.ActivationFunctionType
    ALU = mybir.AluOpType

    calls = prep["calls"]
    tasks = prep["tasks"]
    chunk_tasks = prep["chunk_tasks"]
    nchunk = prep["nchunk"]
    nslot = prep["nslot"]
    ntask = prep["ntask"]

    # last task index per block (for PSUM stop flags)
    last_task_of_blk = {}
    for t, (j, b) in enumerate(tasks):
        last_task_of_blk[b] = t
    max_call_ch = max(n for _, _, _, _, n in calls)

    nc = bacc.Bacc("TRN2", target_bir_lowering=False, debug=debug, num_swdge_queues=cfg.NQ, dynamic_dma_scratch_size=32768)

    xT_in = nc.dram_tensor("xT", [cfg.IN_C, cfg.NPC_PAD], bf16, kind="ExternalInput")
    W1_in = nc.dram_tensor("W1", [cfg.IN_C, cfg.HID], bf16, kind="ExternalInput")
    W2_in = nc.dram_tensor("W2", [cfg.HID, cfg.OUT_C], bf16, kind="ExternalInput")
    b1r_in = nc.dram_tensor("b1rep", [cfg.BLK, cfg.HID], fp32, kind="ExternalInput")
    b2c_in = nc.dram_tensor("b2col", [cfg.OUT_C, 1], fp32, kind="ExternalInput")
    dinv_in = nc.dram_tensor("dinvcol", [cfg.BLK, cfg.NBLK], fp32, kind="ExternalInput")
    ident_in = nc.dram_tensor("ident", [cfg.BLK, cfg.BLK], bf16, kind="ExternalInput")
    iota_in = nc.dram_tensor("iota", [cfg.BLK, cfg.SG * cfg.BLK], fp32, kind="ExternalInput")
    idx_in = nc.dram_tensor("idxs", [128, nslot // 16], mybir.dt.int16, kind="ExternalInput")
    dstloc_in = nc.dram_tensor("dstloc", [cfg.BLK, ntask], fp32, kind="ExternalInput")
    out_t = nc.dram_tensor("outT", [cfg.OUT_C, cfg.NPC_PAD], bf16, kind="ExternalOutput")

    shard1h = [nc.dram_tensor(f"shard1{h}", [cfg.HALF, cfg.HID], bf16) for h in range(2)]
    shard2h = [nc.dram_tensor(f"shard2{h}", [cfg.HALF, cfg.HID], bf16) for h in range(2)]
    table1h = [
        nc.dram_tensor(f"table1{h}", [cfg.TABH, cfg.HID], bf16, addr_space="Shared")
        for h in range(2)
    ]
    table2h = [
        nc.dram_tensor(f"table2{h}", [cfg.TABH, cfg.HID], bf16, addr_space="Shared")
        for h in range(2)
    ]

    replica = [list(range(cfg.CORES))]
    npairs_h = cfg.TABH // 2

    with tile.TileContext(nc) as tc:
        with (
            tc.tile_pool(name="cst", bufs=1) as cst,
            tc.tile_pool(name="stg", bufs=1) as stg,
            tc.tile_pool(name="gp", bufs=10) as gp,
            tc.tile_pool(name="sp", bufs=10) as sp,
            tc.tile_pool(name="ev", bufs=6) as ev,
        ):
            nc.gpsimd.load_library(library_config.mlp)

            # ---- constants ----
            W1t = cst.tile([cfg.IN_C, cfg.HID], bf16)
            nc.sync.dma_start(W1t[:], W1_in[:])
            W2t = cst.tile([cfg.HID, cfg.OUT_C], bf16)
            nc.sync.dma_start(W2t[:], W2_in[:])
            b1rt = cst.tile([cfg.BLK, cfg.HID], fp32)
            nc.sync.dma_start(b1rt[:], b1r_in[:])
            b2t = cst.tile([cfg.OUT_C, 1], fp32)
            nc.sync.dma_start(b2t[:], b2c_in[:])
            dinvt = cst.tile([cfg.BLK, cfg.NBLK], fp32)
            nc.sync.dma_start(dinvt[:], dinv_in[:])
            identt = cst.tile([cfg.BLK, cfg.BLK], bf16)
            nc.sync.dma_start(identt[:], ident_in[:])
            iota = cst.tile([cfg.BLK, cfg.SG * cfg.BLK], fp32)
            nc.sync.dma_start(iota[:], iota_in[:])
            idxt = cst.tile([128, nslot // 16], mybir.dt.int16)
            nc.sync.dma_start(idxt[:], idx_in[:])
            dstloct = cst.tile([cfg.BLK, ntask], fp32)
            nc.sync.dma_start(dstloct[:], dstloc_in[:])

            stag1a = stg.tile([cfg.BLK, cfg.NBH, cfg.HID], bf16)
            stag1b = stg.tile([cfg.BLK, cfg.NBH, cfg.HID], bf16)
            stag2a = stg.tile([cfg.BLK, cfg.NBH, cfg.HID], bf16)
            stag2b = stg.tile([cfg.BLK, cfg.NBH, cfg.HID], bf16)
            stag1h = [stag1a, stag1b]
            stag2h = [stag2a, stag2b]
            outT = stg.tile([cfg.OUT_C, cfg.NPC_PAD], bf16)

            def stag_slice(tiles, b):
                h, bl = (0, b) if b < cfg.NBH else (1, b - cfg.NBH)
                return tiles[h][:, bl : bl + 1, :]

            # ---- layer-1 transform: stag1 = dinv * (x @ W1) ----
            with (
                tc.tile_pool(name="phA", bufs=1) as pa,
                tc.tile_pool(name="psA", bufs=4, space="PSUM") as psA,
            ):
                xtile = pa.tile([cfg.IN_C, cfg.NPC_PAD], bf16)
                nc.sync.dma_start(xtile[:], xT_in[:])
                for b in range(cfg.NBLK):
                    ps = psA.tile([cfg.BLK, cfg.HID], fp32)
                    nc.tensor.matmul(
                        ps[:],
                        lhsT=xtile[:, b * cfg.BLK : (b + 1) * cfg.BLK],
                        rhs=W1t[:],
                        start=True,
                        stop=True,
                    )
                    nc.vector.tensor_tensor(
                        out=stag_slice(stag1h, b),
                        in0=ps[:].rearrange("p (a d) -> p a d", a=1),
                        in1=dinvt[:, b : b + 1].to_broadcast([cfg.BLK, 1, cfg.HID]),
                        op=ALU.mult,
                    )
                for h in range(2):
                    nc.sync.dma_start(
                        shard1h[h].rearrange("(b p) d -> p b d", p=cfg.BLK)[:],
                        stag1h[h][:],
                    )
                    nc.gpsimd.collective_compute(
                        "AllGather",
                        mybir.AluOpType.bypass,
                        replica_groups=replica,
                        ins=[shard1h[h][:]],
                        outs=[table1h[h][:]],
                    )

            # ---- aggregation (shared for both layers) ----
            def agg_layer(layer, tables, stag_tiles, epilogue, after_sb=None):
                pairs_h = [t.rearrange("(r two) d -> r (two d)", two=2) for t in tables]
                s_tiles = {}

                def s_for(t):
                    gi = t // cfg.SG
                    if gi not in s_tiles:
                        n = min(cfg.SG, ntask - gi * cfg.SG)
                        st = sp.tile([128, cfg.SG * cfg.BLK], bf16, tag="s")
                        nc.vector.tensor_tensor(
                            out=st[:].rearrange("p (a b) -> p a b", b=cfg.BLK)[:, :n, :],
                            in0=iota[:].rearrange("p (a b) -> p a b", b=cfg.BLK)[
                                :, :n, :
                            ],
                            in1=dstloct[:, gi * cfg.SG : gi * cfg.SG + n].to_broadcast(
                                [128, n, cfg.BLK]
                            ),
                            op=ALU.is_equal,
                        )
                        s_tiles[gi] = st
                    return s_tiles[gi], (t % cfg.SG)

                for sb in range(cfg.NSB):
                    blo = sb * cfg.SBB
                    bhi = min((sb + 1) * cfg.SBB, cfg.NBLK)
                    psums = {}
                    for b in range(blo, bhi):
                        ps = tc_psum.tile([cfg.BLK, cfg.HID], fp32, tag=f"ps{layer}")
                        psums[b] = ps
                        nc.tensor.matmul(
                            ps[:],
                            lhsT=identt[:],
                            rhs=stag_slice(stag_tiles, b).squeeze(1),
                            start=True,
                            stop=(last_task_of_blk.get(b) is None),
                        )
                    # gather + aggregate this superblock's calls
                    for ci, (call_sb, g, pa_, j0, nch) in enumerate(calls):
                        if call_sb != sb:
                            continue
                        gt = gp.tile([128, max_call_ch, 2 * cfg.HID], bf16, tag="g")
                        nc.gpsimd.dma_gather(
                            gt[:, :nch, :],
                            pairs_h[g][0:npairs_h, :],
                            idxt[:, (j0 * cfg.BLK) // 16 : ((j0 + nch) * cfg.BLK) // 16],
                            nch * cfg.BLK,
                            nch * cfg.BLK,
                            2 * cfg.HID,
                            single_packet=False,
                            queue_num=ci % cfg.NQ,
                        )
                        half = slice(pa_ * cfg.HID, (pa_ + 1) * cfg.HID)
                        for j in range(j0, j0 + nch):
                            for t in chunk_tasks[j]:
                                _, b = tasks[t]
                                st, k = s_for(t)
                                nc.tensor.matmul(
                                    psums[b][:],
                                    lhsT=st[:, k * cfg.BLK : (k + 1) * cfg.BLK],
                                    rhs=gt[:, j - j0, half],
                                    start=False,
                                    stop=(t == last_task_of_blk[b]),
                                )
                    for b in range(blo, bhi):
                        epilogue(b, psums.pop(b))
                    if after_sb:
                        for fn in after_sb.get(sb, []):
                            fn()

            # layer 1 aggregation
            def epi1(b, ps):
                u = ev.tile([cfg.BLK, cfg.HID], fp32, tag="u")
                nc.vector.tensor_tensor(
                    out=u[:].rearrange("p (a d) -> p a d", a=1),
                    in0=ps[:].rearrange("p (a d) -> p a d", a=1),
                    in1=dinvt[:, b : b + 1].to_broadcast([cfg.BLK, 1, cfg.HID]),
                    op=ALU.mult,
                )
                v = ev.tile([cfg.BLK, cfg.HID], fp32, tag="v")
                nc.vector.tensor_tensor(out=v[:], in0=u[:], in1=b1rt[:], op=ALU.add)
                w = ev.tile([cfg.BLK, cfg.HID], fp32, tag="w")
                nc.scalar.activation(w[:], v[:], AF.Relu)
                nc.vector.tensor_tensor(
                    out=stag_slice(stag2h, b),
                    in0=w[:].rearrange("p (a d) -> p a d", a=1),
                    in1=dinvt[:, b : b + 1].to_broadcast([cfg.BLK, 1, cfg.HID]),
                    op=ALU.mult,
                )

            def emit_ag2(h):
                def f():
                    nc.sync.dma_start(
                        shard2h[h].rearrange("(b p) d -> p b d", p=cfg.BLK)[:],
                        stag2h[h][:],
                    )
                    nc.gpsimd.collective_compute(
                        "AllGather",
                        mybir.AluOpType.bypass,
                        replica_groups=replica,
                        ins=[shard2h[h][:]],
                        outs=[table2h[h][:]],
                    )

                return f

            # trigger each half's AllGather as soon as its epilogues finish,
            # so the in-order Pool queue overlaps it with later gathers
            sbA = (cfg.NBH - 1) // cfg.SBB
            hooks = {}
            hooks.setdefault(sbA, []).append(emit_ag2(0))
            hooks.setdefault(cfg.NSB - 1, []).append(emit_ag2(1))
            with tc.tile_pool(name="psagg1", bufs=cfg.SBB, space="PSUM") as tc_psum:
                agg_layer(1, table1h, stag1h, epi1, after_sb=hooks)

            # layer 2 aggregation -> transposed output
            def epi2(b, ps):
                z = ev.tile([cfg.BLK, cfg.HID], bf16, tag="z")
                nc.vector.tensor_tensor(
                    out=z[:].rearrange("p (a d) -> p a d", a=1),
                    in0=ps[:].rearrange("p (a d) -> p a d", a=1),
                    in1=dinvt[:, b : b + 1].to_broadcast([cfg.BLK, 1, cfg.HID]),
                    op=ALU.mult,
                )
                pst = tc_pt.tile([cfg.HID, cfg.BLK], fp32, tag="pt")
                nc.tensor.matmul(pst[:], lhsT=z[:], rhs=identt[:], start=True, stop=True)
                zt = ev.tile([cfg.HID, cfg.BLK], bf16, tag="zt")
                nc.scalar.activation(zt[:], pst[:], AF.Copy)
                pso = tc_po.tile([cfg.OUT_C, cfg.BLK], fp32, tag="po")
                nc.tensor.matmul(pso[:], lhsT=W2t[:], rhs=zt[:], start=True, stop=True)
                nc.vector.tensor_tensor(
                    out=outT[:, b * cfg.BLK : (b + 1) * cfg.BLK].rearrange(
                        "p (a d) -> p a d", a=1
                    ),
                    in0=pso[:].rearrange("p (a d) -> p a d", a=1),
                    in1=b2t[:, 0:1].to_broadcast([cfg.OUT_C, 1, cfg.BLK]),
                    op=ALU.add,
                )

            with (
                tc.tile_pool(name="psagg2", bufs=cfg.SBB, space="PSUM") as tc_psum,
                tc.tile_pool(name="pspt", bufs=1, space="PSUM") as tc_pt,
                tc.tile_pool(name="pspo", bufs=1, space="PSUM") as tc_po,
            ):
                agg_layer(2, table2h, stag2h, epi2)

            nc.sync.dma_start(out_t[:], outT[:])

    nc.compile()
    return nc


# ---------------------------------------------------------------------------
# public entry point
# ---------------------------------------------------------------------------
def _make_in_maps(cfg, prep, x, W1, b1, W2, b2):
    dinv = prep["dinv"]
    iota = np.tile(np.arange(cfg.BLK, dtype=np.float32), (cfg.BLK, cfg.SG)).reshape(
        cfg.BLK, cfg.SG * cfg.BLK
    )
    in_maps = []
    for c in range(cfg.CORES):
        xs = np.asarray(x[c * cfg.NPC : (c + 1) * cfg.NPC], np.float32)
        xT = np.zeros((cfg.IN_C, cfg.NPC_PAD), np.float32)
        xT[:, : cfg.NPC] = xs.T
        dv = np.zeros(cfg.NPC_PAD, np.float32)
        dv[: cfg.NPC] = dinv[c * cfg.NPC : (c + 1) * cfg.NPC]
        in_maps.append(
            {
                "xT": xT.astype(BF16),
                "W1": np.asarray(W1, np.float32).astype(BF16),
                "W2": np.asarray(W2, np.float32).astype(BF16),
                "b1rep": np.tile(np.asarray(b1, np.float32), (cfg.BLK, 1)),
                "b2col": np.asarray(b2, np.float32).reshape(cfg.OUT_C, 1),
                "dinvcol": np.ascontiguousarray(
                    dv.reshape(cfg.NBLK, cfg.BLK).T
                ),
                "ident": np.eye(cfg.BLK, dtype=np.float32).astype(BF16),
                "iota": iota.astype(np.float32),
                "idxs": prep["idx_maps"][c],
                "dstloc": prep["dstloc_maps"][c],
            }
        )
    return in_maps


def _run(cfg, inputs, mode="hw", trace=False, taps=False):
    x = np.asarray(inputs["x"], np.float32)
    edge_index = np.asarray(inputs["edge_index"])
    W1 = np.asarray(inputs["W1"], np.float32)
    b1 = np.asarray(inputs["b1"], np.float32)
    W2 = np.asarray(inputs["W2"], np.float32)
    b2 = np.asarray(inputs["b2"], np.float32)

    prep = _prepare(cfg, edge_index)
    nc = _build(cfg, prep, debug=(mode == "sim"), taps=taps)
    in_maps = _make_in_maps(cfg, prep, x, W1, b1, W2, b2)

    info = {}
    if mode == "sim":
        from concourse.bass_interp import MultiCoreSim

        sim = MultiCoreSim(nc, cfg.CORES)
        for c in range(cfg.CORES):
            for k, v in in_maps[c].items():
                sim.cores[c].tensor(k)[:] = v
        sim.simulate()
        outs = [sim.cores[c].tensor("outT").copy() for c in range(cfg.CORES)]
        if taps:
            info["taps"] = [
                {k: sim.cores[c].tensor(k).copy() for k in ("tap1", "tap2")}
                for c in range(cfg.CORES)
            ]
    else:
        import concourse.bass_utils as bu

        if trace:
            bu.upload_artifacts = lambda d: "(local)"
        r = bu.run_bass_kernel_spmd(
            nc, in_maps, list(range(cfg.CORES)), trace=trace,
            tmpdir=(inputs.get("_tracedir") if trace else None),
        )
        info["exec_time_ns"] = r.exec_time_ns
        info["mean_exec_time_ns"] = r.mean_exec_time_ns
        outs = [r.results[c]["outT"] for c in range(cfg.CORES)]
        if taps:
            info["taps"] = [
                {k: r.results[c][k] for k in ("tap1", "tap2")} for c in range(cfg.CORES)
            ]

    out = np.concatenate(
        [np.asarray(o, dtype=np.float32)[:, : cfg.NPC].T for o in outs], axis=0
    )
    return out.astype(np.float32), info


def kernel(**inputs):
    out, _ = _run(Cfg(), inputs, mode="hw")
    return out
